# revision 14
# baseline (speedup 1.0000x reference)
"""Trainium2 Bass kernel for the cluster-GRU salience model.

Model (see reference): sentences are grouped by cluster label, each cluster's
sentence sequence is run through a 2-layer GRU, the final hidden state is
scattered back onto the cluster's sentences, scored through a weight-normed
linear + tanh, normalized by per-cluster segment sums, and mixed with a
positional score.

Strategy:
  - Host groups the N=4096 sentences by cluster (K=32), assigns 4 clusters to
    each of the 8 cores, and LEFT-pads every cluster sequence to the global
    max length T so all clusters finish at the same slot (uniform SPMD
    program).  Pad steps keep h frozen by forcing the update gate z to
    sigmoid(+30) ~= 1 via host-built selector matrices that feed a tiny
    bias matmul.
  - On device, both GRU layers run fused per slot (layer 2 lags DELTA slots),
    with gates on partitions ([H=128, B=4] tiles).  All xw + hw + bias adds
    are folded into PSUM matmul accumulation; the per-slot chain is
    sigmoid -> (r*hw_n) -> (+xw_n) -> tanh -> (1-z)*n -> +z*h.
  - Layer-2's input transform (W_ih1 @ h1) is computed in bulk every 8 slots.
  - Scoring reuses the on-chip transposed embeddings: a [1 x cols] matvec,
    per-cluster beta add, tanh, masked per-cluster segment sums, and a fused
    (score * 0.5/sum + pos/2) epilogue.
"""

import os
import sys

import numpy as np

for _p in ("/opt/trn_rl_repo",):
    if _p not in sys.path and os.path.isdir(_p):
        sys.path.insert(0, _p)

N = 4096
K = 32
D = 256
H = 128
P_SAL = 0.5
NCORES = 8
B = K // NCORES          # clusters per core
DELTA = 12               # layer-2 lag (slots)
XCHUNK = 8               # slots per bulk xw2 chunk
ZPAD = 30.0              # pad-step z-gate pre-activation (sigmoid(30) ~ 1)
FILLERS = 5              # bf16 junk matmuls per slot to keep PE HAM warm
FILLN = 512              # filler moving columns


def _build_program(T, S):
    """Build + compile the (shared, SPMD) Bass program.

    T: global max cluster length; S = T + DELTA total recurrence slots.
    All per-core variation lives in the input data, not the program.
    """
    import concourse.bacc as bacc
    import concourse.mybir as mybir
    import concourse.tile as tile

    f32 = mybir.dt.float32
    Alu = mybir.AluOpType
    Act = mybir.ActivationFunctionType

    SB = S * B
    B2, B4, B6 = 2 * B, 4 * B, 6 * B

    nc = bacc.Bacc("TRN2", target_bir_lowering=False, debug=False,
                   num_devices=NCORES)

    # ---- DRAM tensors (per-core inputs) ----
    # x_pack: transposed, cluster-major padded embeddings.
    #   [:, 0:SB]      = features   0:128 at col t*B+b
    #   [:, SB:2*SB]   = features 128:256 at col t*B+b
    x_dram = nc.dram_tensor("x_pack", [128, 2 * SB], f32, kind="ExternalInput")
    # wpack columns: Wih0T lo,hi (2*384) | Whh0T (384) | Wih1T (384) |
    #                Whh1T (384) | ident (128) | ws lo,hi (2) | wh (1) | bhn0|bhn1 (2)
    WCOLS = 2 * 384 + 3 * 384 + 128 + 5
    w_dram = nc.dram_tensor("w_pack", [128, WCOLS], f32, kind="ExternalInput")
    # spack: [2, x] selector/bias rows:
    #   sel1 (SB) | sel2 (T*B) | selnh (2B) | B1 (384) | B2 (384) | bhn (128)
    SCOLS = SB + T * B + B2 + 384 + 384 + 128
    s_dram = nc.dram_tensor("s_pack", [2, SCOLS], f32, kind="ExternalInput")
    # rpack: [1, x]: pos_half (SB) | mask (SB) | lin_b (1)
    r_dram = nc.dram_tensor("r_pack", [1, 2 * SB + 1], f32, kind="ExternalInput")

    out_dram = nc.dram_tensor("scores", [1, SB], f32, kind="ExternalOutput")

    with tile.TileContext(nc) as tc:
        with (
            tc.tile_pool(name="persist", bufs=1) as pp,
            tc.tile_pool(name="work", bufs=3) as wp,
            tc.tile_pool(name="ps", bufs=2, space="PSUM") as ps,
            tc.tile_pool(name="psj", bufs=1, space="PSUM") as psj,
            tc.tile_pool(name="psbulk", bufs=2, space="PSUM") as psb,
        ):
            # ---- load inputs into SBUF ----
            x_sb = pp.tile([128, 2 * SB], f32, tag="x")
            w_sb = pp.tile([128, WCOLS], f32, tag="w")
            s_sb = pp.tile([2, SCOLS], f32, tag="s")
            r_sb = pp.tile([1, 2 * SB + 1], f32, tag="r")
            nc.sync.dma_start(x_sb[:], x_dram.ap()[:])
            nc.sync.dma_start(w_sb[:], w_dram.ap()[:])
            nc.sync.dma_start(s_sb[:], s_dram.ap()[:])
            nc.sync.dma_start(r_sb[:], r_dram.ap()[:])

            # named views of the packs
            Wih0_lo = lambda g: w_sb[:, g * 128:(g + 1) * 128]
            Wih0_hi = lambda g: w_sb[:, 384 + g * 128:384 + (g + 1) * 128]
            Whh0 = lambda g: w_sb[:, 768 + g * 128:768 + (g + 1) * 128]
            Wih1 = lambda g: w_sb[:, 1152 + g * 128:1152 + (g + 1) * 128]
            Whh1 = lambda g: w_sb[:, 1536 + g * 128:1536 + (g + 1) * 128]
            ident = w_sb[:, 1920:2048]
            ws_lo = w_sb[:, 2048:2049]
            ws_hi = w_sb[:, 2049:2050]
            wh = w_sb[:, 2050:2051]
            bhn0 = w_sb[:, 2051:2052]
            bhn1 = w_sb[:, 2052:2053]

            o_sel2 = SB
            o_selnh = SB + T * B
            o_B1 = o_selnh + B2
            o_B2 = o_B1 + 384
            o_bhn = o_B2 + 384
            sel1 = s_sb[:, 0:SB]
            sel2 = s_sb[:, o_sel2:o_sel2 + T * B]
            selnh = s_sb[:, o_selnh:o_selnh + B2]
            B1g = lambda g: s_sb[:, o_B1 + g * 128:o_B1 + (g + 1) * 128]
            B2g = lambda g: s_sb[:, o_B2 + g * 128:o_B2 + (g + 1) * 128]
            bhnmat = s_sb[:, o_bhn:o_bhn + 128]

            pos_half = r_sb[:, 0:SB]
            rmask = r_sb[:, SB:2 * SB]
            lin_b = r_sb[:, 2 * SB:2 * SB + 1]

            # ---- persistent state / stores ----
            # rz_store[:, s, :]: a1_r | xw2_r | a1_z | xw2_z
            rz_store = pp.tile([128, S, B4], f32, tag="rzs")
            # nx_store[:, s, :]: a1_n | xw2_n
            nx_store = pp.tile([128, S, B2], f32, tag="nxs")
            # hist[:, s, :]: [h1 | h2] state BEFORE slot s
            hist = pp.tile([128, S + 1, B2], f32, tag="hist")
            # junk operand for PE warm-up fillers
            jnk = pp.tile([128, FILLN], mybir.dt.bfloat16, tag="jnk")
            nc.vector.memset(jnk[:], 0.0)

            nc.vector.memset(hist[:, 0:1, :], 0.0)
            # layer-2 parts of store slots [0, DELTA): force z2 pad
            nc.vector.memset(rz_store[:, 0:DELTA, B:B2], 0.0)
            nc.vector.memset(rz_store[:, 0:DELTA, 3 * B:B4], ZPAD)
            nc.vector.memset(nx_store[:, 0:DELTA, B:B2], 0.0)

            # ---- phase A: a1 = W_ih0 @ x (+ biases via selector MM) ----
            # chunks of 128 slots (512 cols)
            t0 = 0
            while t0 < S:
                t1 = min(t0 + 128, S)
                cols = (t1 - t0) * B
                c0 = t0 * B
                for g in range(3):
                    pa = psb.tile([128, 512], f32, tag="bulk")
                    nc.tensor.matmul(out=pa[:, 0:cols], lhsT=B1g(g),
                                     rhs=sel1[:, c0:c0 + cols],
                                     start=True, stop=False)
                    nc.tensor.matmul(out=pa[:, 0:cols], lhsT=Wih0_lo(g),
                                     rhs=x_sb[:, c0:c0 + cols],
                                     start=False, stop=False)
                    nc.tensor.matmul(out=pa[:, 0:cols], lhsT=Wih0_hi(g),
                                     rhs=x_sb[:, SB + c0:SB + c0 + cols],
                                     start=False, stop=True)
                    src = pa[:, 0:cols].rearrange("p (t b) -> p t b", b=B)
                    if g == 0:
                        nc.vector.tensor_copy(out=rz_store[:, t0:t1, 0:B], in_=src)
                    elif g == 1:
                        nc.vector.tensor_copy(out=rz_store[:, t0:t1, 2 * B:3 * B],
                                              in_=src)
                    else:
                        nc.vector.tensor_copy(out=nx_store[:, t0:t1, 0:B], in_=src)
                t0 = t1

            # ---- recurrence ----
            # warm burst so the PE HAM doesn't re-throttle at the phase
            # boundary before the slot loop's own fillers take over
            for _ in range(16):
                pj = psj.tile([1, FILLN], f32, tag="junk")
                nc.tensor.matmul(out=pj[:], lhsT=jnk[:, 0:1],
                                 rhs=jnk[:], start=True, stop=True)

            nchunks = (T + XCHUNK - 1) // XCHUNK
            next_chunk = 0

            for s in range(S):
                h_prev = hist[:, s, :]          # [128, 2B] = [h1 | h2]
                h1_prev = hist[:, s, 0:B]
                h2_prev = hist[:, s, B:B2]

                # PSUM bank A: [r1 r2 z1 z2]; bank B: [n1h n2h] (+ b_hh_n)
                pg = ps.tile([128, B4], f32, tag="gates")
                nc.tensor.matmul(out=pg[:], lhsT=ident, rhs=rz_store[:, s, :],
                                 start=True, stop=False)
                nc.tensor.matmul(out=pg[:, 0:B], lhsT=Whh0(0), rhs=h1_prev,
                                 start=False, stop=False)
                nc.tensor.matmul(out=pg[:, B:B2], lhsT=Whh1(0), rhs=h2_prev,
                                 start=False, stop=False)
                nc.tensor.matmul(out=pg[:, B2:3 * B], lhsT=Whh0(1), rhs=h1_prev,
                                 start=False, stop=False)
                nc.tensor.matmul(out=pg[:, 3 * B:B4], lhsT=Whh1(1), rhs=h2_prev,
                                 start=False, stop=True)
                pn = ps.tile([128, B2], f32, tag="nh")
                nc.tensor.matmul(out=pn[:], lhsT=bhnmat, rhs=selnh,
                                 start=True, stop=False)
                nc.tensor.matmul(out=pn[:, 0:B], lhsT=Whh0(2), rhs=h1_prev,
                                 start=False, stop=False)
                nc.tensor.matmul(out=pn[:, B:B2], lhsT=Whh1(2), rhs=h2_prev,
                                 start=False, stop=True)
                for _ in range(FILLERS):
                    pj = psj.tile([1, FILLN], f32, tag="junk")
                    nc.tensor.matmul(out=pj[:], lhsT=jnk[:, 0:1],
                                     rhs=jnk[:], start=True, stop=True)

                sig = wp.tile([128, B4], f32, tag="sig")
                nc.scalar.activation(sig[:], pg[:], Act.Sigmoid)
                w1z = wp.tile([128, B2], f32, tag="w1z")
                nc.scalar.activation(w1z[:], pg[:, B2:B4], Act.Sigmoid,
                                     scale=-1.0)
                tn = wp.tile([128, B2], f32, tag="tn")
                nc.vector.tensor_tensor(out=tn[:], in0=sig[:, 0:B2],
                                        in1=pn[:], op=Alu.mult)
                tn2 = wp.tile([128, B2], f32, tag="tn2")
                nc.vector.tensor_tensor(out=tn2[:], in0=tn[:],
                                        in1=nx_store[:, s, :], op=Alu.add)
                nt = wp.tile([128, B2], f32, tag="nt")
                nc.scalar.activation(nt[:], tn2[:], Act.Tanh)
                vzh = wp.tile([128, B2], f32, tag="vzh")
                nc.vector.tensor_tensor(out=vzh[:], in0=sig[:, B2:B4],
                                        in1=h_prev, op=Alu.mult)
                uwn = wp.tile([128, B2], f32, tag="uwn")
                nc.vector.tensor_tensor(out=uwn[:], in0=w1z[:], in1=nt[:],
                                        op=Alu.mult)
                nc.vector.tensor_tensor(out=hist[:, s + 1, :], in0=uwn[:],
                                        in1=vzh[:], op=Alu.add)

                # bulk xw2 chunk: after slot 8c+7 (h1 up to t'=8c+7 written)
                if next_chunk < nchunks and s == min(
                        XCHUNK * next_chunk + XCHUNK - 1, S - 1):
                    c = next_chunk
                    tp0 = XCHUNK * c
                    tp1 = min(tp0 + XCHUNK, T)
                    ccols = (tp1 - tp0) * B
                    px = psb.tile([128, 3, XCHUNK * B], f32, tag="bulk")
                    h1c = hist[:, tp0 + 1:tp1 + 1, 0:B]
                    for g in range(3):
                        nc.tensor.matmul(
                            out=px[:, g, 0:ccols], lhsT=B2g(g),
                            rhs=sel2[:, tp0 * B:tp0 * B + ccols],
                            start=True, stop=False)
                        nc.tensor.matmul(out=px[:, g, 0:ccols], lhsT=Wih1(g),
                                         rhs=h1c, start=False, stop=True)
                    so0 = tp0 + DELTA
                    so1 = tp1 + DELTA
                    for g, (store, col) in enumerate(
                            ((rz_store, B), (rz_store, 3 * B), (nx_store, B))):
                        nc.vector.tensor_copy(
                            out=store[:, so0:so1, col:col + B],
                            in_=px[:, g, 0:ccols].rearrange(
                                "p (t b) -> p t b", b=B))
                    next_chunk += 1

            # ---- scoring ----
            h2f = hist[:, S, B:B2]                     # final layer-2 states
            pbeta_t = psb.tile([1, 512], f32, tag="bulk")
            pbeta = pbeta_t[:, 0:B]
            nc.tensor.matmul(out=pbeta[:], lhsT=wh, rhs=h2f,
                             start=True, stop=True)
            beta = wp.tile([1, B], f32, tag="betasb")
            nc.vector.tensor_copy(out=beta[:], in_=pbeta[:])

            sc = pp.tile([1, S, B], f32, tag="sc")
            c0 = 0
            while c0 < SB:
                c1 = min(c0 + 512, SB)
                pscore = psb.tile([1, 512], f32, tag="bulk")
                nc.tensor.matmul(out=pscore[:, 0:c1 - c0], lhsT=ws_lo,
                                 rhs=x_sb[:, c0:c1], start=True, stop=False)
                nc.tensor.matmul(out=pscore[:, 0:c1 - c0], lhsT=ws_hi,
                                 rhs=x_sb[:, SB + c0:SB + c1],
                                 start=False, stop=True)
                nc.vector.tensor_copy(
                    out=sc[:].rearrange("p t b -> p (t b)")[:, c0:c1],
                    in_=pscore[:, 0:c1 - c0])
                c0 = c1

            # + beta[cluster], tanh(. + lin_b), mask, segment sums
            for b in range(B):
                nc.vector.tensor_scalar_add(out=sc[:, :, b:b + 1],
                                            in0=sc[:, :, b:b + 1],
                                            scalar1=beta[:, b:b + 1])
            th = pp.tile([1, S, B], f32, tag="th")
            nc.scalar.activation(th[:], sc[:], Act.Tanh, bias=lin_b)
            nc.vector.tensor_tensor(
                out=th[:], in0=th[:],
                in1=rmask.rearrange("p (t b) -> p t b", b=B), op=Alu.mult)
            sums = wp.tile([1, B], f32, tag="sums")
            for b in range(B):
                nc.vector.tensor_reduce(out=sums[:, b:b + 1],
                                        in_=th[:, :, b:b + 1],
                                        axis=mybir.AxisListType.XY, op=Alu.add)
            rsum = wp.tile([1, B], f32, tag="rsum")
            nc.vector.reciprocal(out=rsum[:], in_=sums[:])
            shalf = wp.tile([1, B], f32, tag="shalf")
            nc.vector.tensor_scalar_mul(out=shalf[:], in0=rsum[:],
                                        scalar1=P_SAL)
            fin = pp.tile([1, S, B], f32, tag="fin")
            for b in range(B):
                nc.vector.scalar_tensor_tensor(
                    out=fin[:, :, b:b + 1], in0=th[:, :, b:b + 1],
                    scalar=shalf[:, b:b + 1],
                    in1=pos_half.rearrange("p (t b) -> p t b", b=B)[:, :, b:b + 1],
                    op0=Alu.mult, op1=Alu.add)

            nc.sync.dma_start(out_dram.ap()[:],
                              fin[:].rearrange("p t b -> p (t b)"))

    nc.compile()
    return nc


def _prep_host(inputs):
    """Host-side sharding/packing.  Returns (T, S, in_maps, scatter)."""
    sent = np.ascontiguousarray(inputs["sent_gae_embeds"], dtype=np.float32)
    labels = np.asarray(inputs["labels"]).astype(np.int64)
    W_ih0 = np.asarray(inputs["W_ih0"], dtype=np.float32)
    W_hh0 = np.asarray(inputs["W_hh0"], dtype=np.float32)
    b_ih0 = np.asarray(inputs["b_ih0"], dtype=np.float32)
    b_hh0 = np.asarray(inputs["b_hh0"], dtype=np.float32)
    W_ih1 = np.asarray(inputs["W_ih1"], dtype=np.float32)
    W_hh1 = np.asarray(inputs["W_hh1"], dtype=np.float32)
    b_ih1 = np.asarray(inputs["b_ih1"], dtype=np.float32)
    b_hh1 = np.asarray(inputs["b_hh1"], dtype=np.float32)
    lin_v = np.asarray(inputs["lin_v"], dtype=np.float32)
    lin_g = np.asarray(inputs["lin_g"], dtype=np.float32)
    lin_b = np.asarray(inputs["lin_b"], dtype=np.float32)

    idx = [np.flatnonzero(labels == k) for k in range(K)]
    lens = np.array([max(len(i), 1) for i in idx])
    T = int(lens.max())
    S = T + DELTA
    SB = S * B

    # weight-normed linear
    Wn = (lin_g * lin_v / np.linalg.norm(lin_v, axis=1, keepdims=True))[0]
    w_s, w_h = Wn[:D], Wn[D:]

    # position scores (host constant table, permuted to cluster layout)
    inv = np.float32(1.0 / (N ** (1.0 / 3.0)))
    pos_full = np.maximum(np.float32(0.5),
                          np.exp(-(np.arange(N, dtype=np.float32) + 1.0) * inv))

    # shared weight pack
    WCOLS = 2 * 384 + 3 * 384 + 128 + 5
    wpack = np.zeros((128, WCOLS), np.float32)
    wihT = W_ih0.T                                   # [256, 384]
    wpack[:, 0:384] = wihT[:128]
    wpack[:, 384:768] = wihT[128:]
    wpack[:, 768:1152] = W_hh0.T
    wpack[:, 1152:1536] = W_ih1.T
    wpack[:, 1536:1920] = W_hh1.T
    wpack[:, 1920:2048] = np.eye(128, dtype=np.float32)
    wpack[:, 2048] = w_s[:128]
    wpack[:, 2049] = w_s[128:]
    wpack[:, 2050] = w_h
    wpack[:, 2051] = b_hh0[2 * H:]
    wpack[:, 2052] = b_hh1[2 * H:]

    SCOLS = SB + T * B + 2 * B + 384 + 384 + 128
    o_sel2 = SB
    o_selnh = SB + T * B
    o_B1 = o_selnh + 2 * B
    o_B2 = o_B1 + 384
    o_bhn = o_B2 + 384

    spack_base = np.zeros((2, SCOLS), np.float32)
    # selnh: row0 -> first B cols (layer1), row1 -> second B cols.  (unused
    # now that bhn columns live in rz_store, but kept for layout stability)
    spack_base[0, o_selnh:o_selnh + B] = 1.0
    spack_base[1, o_selnh + B:o_selnh + 2 * B] = 1.0
    # B1: row0 = real bias (b_ih0 + b_hh0 for r,z; b_ih0 for n); row1 = pad
    breal = b_ih0.copy()
    breal[:2 * H] += b_hh0[:2 * H]
    spack_base[0, o_B1:o_B1 + 384] = breal
    spack_base[1, o_B1 + H:o_B1 + 2 * H] = ZPAD
    breal2 = b_ih1.copy()
    breal2[:2 * H] += b_hh1[:2 * H]
    spack_base[0, o_B2:o_B2 + 384] = breal2
    spack_base[1, o_B2 + H:o_B2 + 2 * H] = ZPAD
    spack_base[0, o_bhn:o_bhn + 128] = b_hh0[2 * H:]
    spack_base[1, o_bhn:o_bhn + 128] = b_hh1[2 * H:]

    in_maps = []
    scatter = []  # per core: list of (orig_index, col) pairs
    for d in range(NCORES):
        xp = np.zeros((128, 2 * SB), np.float32)
        sp = spack_base.copy()
        rp = np.zeros((1, 2 * SB + 1), np.float32)
        sc_pairs = []
        for b in range(B):
            k = d * B + b
            ids = idx[k]
            L = lens[k]
            pad = T - L
            # real slots t in [pad, pad+L); sentence j = t - pad
            cols = (np.arange(pad, pad + L) * B + b)
            if len(ids):
                xp[:, cols] = sent[ids, :128].T
                xp[:, SB + cols] = sent[ids, 128:].T
                rp[0, cols] = (1.0 - P_SAL) * pos_full[ids]
                sc_pairs.append((ids, cols))
            rp[0, SB + cols] = 1.0                       # mask
            sp[0, cols] = 1.0                            # sel1 real
            sp[1, cols] = 0.0
            pads1 = np.concatenate([np.arange(0, pad), np.arange(pad + L, S)])
            sp[1, pads1 * B + b] = 1.0
            # sel2 indexed by t' in [0, T)
            c2 = o_sel2 + np.arange(T) * B + b
            sp[0, c2[pad:pad + L]] = 1.0
            sp[1, c2[:pad]] = 1.0
            sp[1, c2[pad + L:]] = 1.0
        rp[0, 2 * SB] = lin_b[0]
        in_maps.append({"x_pack": xp, "w_pack": wpack, "s_pack": sp,
                        "r_pack": rp})
        scatter.append(sc_pairs)

    return T, S, in_maps, scatter


_PROGRAM_CACHE = {}


def _install_ntff_hook_shim():
    """Provide antenv.axon_hooks (absent in this image) so that
    run_bass_kernel_spmd(trace=True) can capture NTFF profiles via the
    axon PJRT sidechannel.  Bench-only; never used by the grading path."""
    import contextlib
    import ctypes
    import types

    if "antenv.axon_hooks" in sys.modules:
        return
    so_path = "/opt/axon/libaxon_pjrt.so"
    hook = None
    if os.path.exists(so_path):
        lib = ctypes.CDLL(so_path)
        if hasattr(lib, "axon_start_nrt_profile"):
            lib.axon_start_nrt_profile.argtypes = [
                ctypes.POINTER(ctypes.c_int64), ctypes.c_size_t]
            lib.axon_start_nrt_profile.restype = ctypes.c_int64
            lib.axon_stop_nrt_profile.argtypes = [ctypes.c_char_p]
            lib.axon_stop_nrt_profile.restype = ctypes.c_int64

            @contextlib.contextmanager
            def _hook(output_dir, device_ids):
                import jax
                jax.devices()
                if device_ids:
                    ids = (ctypes.c_int64 * len(device_ids))(*device_ids)
                    rc = lib.axon_start_nrt_profile(ids, len(device_ids))
                else:
                    rc = lib.axon_start_nrt_profile(None, 0)
                if rc != 0:
                    raise RuntimeError(f"axon_start_nrt_profile rc={rc}")
                try:
                    yield
                finally:
                    n = lib.axon_stop_nrt_profile(str(output_dir).encode())
                    print(f"profile: {n} file(s) written to {output_dir}",
                          file=sys.stderr)

            hook = _hook

    mod = types.ModuleType("antenv.axon_hooks")
    mod.get_axon_ntff_profile_hook = lambda: hook
    mod.set_axon_ntff_profile_hook = lambda h: None
    sys.modules["antenv.axon_hooks"] = mod


def kernel(_bench=None, **inputs):
    from concourse import bass_utils

    if _bench is not None:
        _install_ntff_hook_shim()

    T, S, in_maps, scatter = _prep_host(inputs)

    key = (T, S)
    if key not in _PROGRAM_CACHE:
        _PROGRAM_CACHE[key] = _build_program(T, S)
    nc = _PROGRAM_CACHE[key]

    res = bass_utils.run_bass_kernel_spmd(
        nc, in_maps, core_ids=list(range(NCORES)),
        trace=_bench is not None, **(_bench or {}))

    out = np.zeros(N, np.float32)
    for d in range(NCORES):
        vals = res.results[d]["scores"][0]
        for ids, cols in scatter[d]:
            out[ids] = vals[cols]

    if _bench is not None:
        kernel._last_results = res
    return out


# revision 19
# speedup vs baseline: 2.7376x; 2.7376x over previous
"""Trainium2 Bass kernel for the cluster-GRU salience model.

Model (see reference): sentences are grouped by cluster label, each cluster's
sentence sequence is run through a 2-layer GRU, the final hidden state is
scattered back onto the cluster's sentences, scored through a weight-normed
linear + tanh, normalized by per-cluster segment sums, and mixed with a
positional score.

Strategy:
  - Host groups the N=4096 sentences by cluster (K=32), assigns 4 clusters to
    each of the 8 cores, and LEFT-pads every cluster sequence to the global
    max length T so all clusters finish at the same slot (uniform SPMD
    program).  Pad steps keep h frozen by forcing the update gate z to
    sigmoid(+30) ~= 1 via host-built selector matrices that feed a tiny
    bias matmul.
  - On device, both GRU layers run fused per slot (layer 2 lags DELTA slots),
    with gates on partitions ([H=128, B=4] tiles).  All xw + hw + bias adds
    are folded into PSUM matmul accumulation; the per-slot chain is
    sigmoid -> (r*hw_n) -> (+xw_n) -> tanh -> (1-z)*n -> +z*h.
  - Layer-2's input transform (W_ih1 @ h1) is computed in bulk every 8 slots.
  - Scoring reuses the on-chip transposed embeddings: a [1 x cols] matvec,
    per-cluster beta add, tanh, masked per-cluster segment sums, and a fused
    (score * 0.5/sum + pos/2) epilogue.
"""

import os
import sys

import numpy as np

for _p in ("/opt/trn_rl_repo",):
    if _p not in sys.path and os.path.isdir(_p):
        sys.path.insert(0, _p)

N = 4096
K = 32
D = 256
H = 128
P_SAL = 0.5
NCORES = 8
B = K // NCORES          # clusters per core
DELTA = 20               # layer-2 lag (slots)
XCHUNK = 16              # slots per bulk xw2 chunk
ZPAD = 30.0              # pad-step z-gate pre-activation (sigmoid(30) ~ 1)


def _build_program(T, S):
    """Build + compile the (shared, SPMD) Bass program.

    T: global max cluster length; S = T + DELTA total recurrence slots.
    All per-core variation lives in the input data, not the program.
    """
    import concourse.bacc as bacc
    import concourse.mybir as mybir
    import concourse.tile as tile

    f32 = mybir.dt.float32
    Alu = mybir.AluOpType
    Act = mybir.ActivationFunctionType

    SB = S * B
    B2, B4, B6 = 2 * B, 4 * B, 6 * B

    nc = bacc.Bacc("TRN2", target_bir_lowering=False, debug=False,
                   num_devices=NCORES)

    # ---- DRAM tensors (per-core inputs) ----
    # x_pack: transposed, cluster-major padded embeddings.
    #   [:, 0:SB]      = features   0:128 at col t*B+b
    #   [:, SB:2*SB]   = features 128:256 at col t*B+b
    x_dram = nc.dram_tensor("x_pack", [128, 2 * SB], f32, kind="ExternalInput")
    # wpack columns: Wih0T lo,hi (2*384) | Whh0T (384) | Wih1T (384) |
    #                Whh1T (384) | ident (128) | ws lo,hi (2) | wh (1) | bhn0|bhn1 (2)
    WCOLS = 2 * 384 + 3 * 384 + 128 + 5
    w_dram = nc.dram_tensor("w_pack", [128, WCOLS], f32, kind="ExternalInput")
    # spack: [2, x] selector/bias rows:
    #   sel1 (SB) | sel2 (T*B) | selnh (2B) | B1 (384) | B2 (384) | bhn (128)
    SCOLS = SB + T * B + B2 + 384 + 384 + 128
    s_dram = nc.dram_tensor("s_pack", [2, SCOLS], f32, kind="ExternalInput")
    # rpack: [1, x]: pos_half (SB) | mask (SB) | lin_b (1)
    r_dram = nc.dram_tensor("r_pack", [1, 2 * SB + 1], f32, kind="ExternalInput")
    f16 = mybir.dt.float16
    # fp16 packs for the per-slot gate matmuls
    w16_dram = nc.dram_tensor("w16_pack", [128, 768], f16, kind="ExternalInput")
    s16_dram = nc.dram_tensor("s16_pack", [2, 128 + B2], f16, kind="ExternalInput")

    out_dram = nc.dram_tensor("scores", [1, SB], f32, kind="ExternalOutput")

    with tile.TileContext(nc) as tc:
        with (
            tc.tile_pool(name="persist", bufs=1) as pp,
            tc.tile_pool(name="work", bufs=3) as wp,
            tc.tile_pool(name="ps", bufs=2, space="PSUM") as ps,
            tc.tile_pool(name="psbulk", bufs=2, space="PSUM") as psb,
        ):
            # ---- load inputs into SBUF ----
            x_sb = pp.tile([128, 2 * SB], f32, tag="x")
            w_sb = pp.tile([128, WCOLS], f32, tag="w")
            s_sb = pp.tile([2, SCOLS], f32, tag="s")
            r_sb = pp.tile([1, 2 * SB + 1], f32, tag="r")
            nc.sync.dma_start(x_sb[:], x_dram.ap()[:])
            nc.sync.dma_start(w_sb[:], w_dram.ap()[:])
            nc.sync.dma_start(s_sb[:], s_dram.ap()[:])
            nc.sync.dma_start(r_sb[:], r_dram.ap()[:])
            w16_sb = pp.tile([128, 768], f16, tag="w16")
            s16_sb = pp.tile([2, 128 + B2], f16, tag="s16")
            nc.scalar.dma_start(w16_sb[:], w16_dram.ap()[:])
            nc.scalar.dma_start(s16_sb[:], s16_dram.ap()[:])
            Whh0_16 = lambda g: w16_sb[:, g * 128:(g + 1) * 128]
            Whh1_16 = lambda g: w16_sb[:, 384 + g * 128:384 + (g + 1) * 128]
            bhnmat16 = s16_sb[:, 0:128]
            selnh16 = s16_sb[:, 128:128 + B2]

            # named views of the packs
            Wih0_lo = lambda g: w_sb[:, g * 128:(g + 1) * 128]
            Wih0_hi = lambda g: w_sb[:, 384 + g * 128:384 + (g + 1) * 128]
            Whh0 = lambda g: w_sb[:, 768 + g * 128:768 + (g + 1) * 128]
            Wih1 = lambda g: w_sb[:, 1152 + g * 128:1152 + (g + 1) * 128]
            Whh1 = lambda g: w_sb[:, 1536 + g * 128:1536 + (g + 1) * 128]
            ident = w_sb[:, 1920:2048]
            ws_lo = w_sb[:, 2048:2049]
            ws_hi = w_sb[:, 2049:2050]
            wh = w_sb[:, 2050:2051]
            bhn0 = w_sb[:, 2051:2052]
            bhn1 = w_sb[:, 2052:2053]

            o_sel2 = SB
            o_selnh = SB + T * B
            o_B1 = o_selnh + B2
            o_B2 = o_B1 + 384
            o_bhn = o_B2 + 384
            sel1 = s_sb[:, 0:SB]
            sel2 = s_sb[:, o_sel2:o_sel2 + T * B]
            selnh = s_sb[:, o_selnh:o_selnh + B2]
            B1g = lambda g: s_sb[:, o_B1 + g * 128:o_B1 + (g + 1) * 128]
            B2g = lambda g: s_sb[:, o_B2 + g * 128:o_B2 + (g + 1) * 128]
            bhnmat = s_sb[:, o_bhn:o_bhn + 128]

            pos_half = r_sb[:, 0:SB]
            rmask = r_sb[:, SB:2 * SB]
            lin_b = r_sb[:, 2 * SB:2 * SB + 1]

            # ---- persistent state / stores ----
            # rz_store[:, s, :]: a1_r | xw2_r | a1_z | xw2_z
            rz_store = pp.tile([128, S, B4], f32, tag="rzs")
            # nx_store[:, s, :]: a1_n | xw2_n
            nx_store = pp.tile([128, S, B2], f32, tag="nxs")
            # hist[:, s, :]: [h1 | h2] state BEFORE slot s (fp32 carry);
            # hist16 is the fp16 shadow feeding the PE gate matmuls
            hist = pp.tile([128, S + 1, B2], f32, tag="hist")
            hist16 = pp.tile([128, S + 1, B2], f16, tag="hist16")
            nc.vector.memset(hist[:, 0:1, :], 0.0)
            nc.vector.memset(hist16[:, 0:1, :], 0.0)
            # layer-2 parts of store slots [0, DELTA): force z2 pad
            nc.vector.memset(rz_store[:, 0:DELTA, B:B2], 0.0)
            nc.vector.memset(rz_store[:, 0:DELTA, 3 * B:B4], -ZPAD)
            nc.vector.memset(nx_store[:, 0:DELTA, B:B2], 0.0)

            # ---- phase A: a1 = W_ih0 @ x (+ biases via selector MM) ----
            # chunks of 128 slots (512 cols)
            t0 = 0
            while t0 < S:
                t1 = min(t0 + 128, S)
                cols = (t1 - t0) * B
                c0 = t0 * B
                for g in range(3):
                    pa = psb.tile([128, 512], f32, tag="bulk")
                    nc.tensor.matmul(out=pa[:, 0:cols], lhsT=B1g(g),
                                     rhs=sel1[:, c0:c0 + cols],
                                     start=True, stop=False)
                    nc.tensor.matmul(out=pa[:, 0:cols], lhsT=Wih0_lo(g),
                                     rhs=x_sb[:, c0:c0 + cols],
                                     start=False, stop=False)
                    nc.tensor.matmul(out=pa[:, 0:cols], lhsT=Wih0_hi(g),
                                     rhs=x_sb[:, SB + c0:SB + c0 + cols],
                                     start=False, stop=True)
                    src = pa[:, 0:cols].rearrange("p (t b) -> p t b", b=B)
                    if g == 0:
                        nc.vector.tensor_copy(out=rz_store[:, t0:t1, 0:B], in_=src)
                    elif g == 1:
                        nc.vector.tensor_copy(out=rz_store[:, t0:t1, 2 * B:3 * B],
                                              in_=src)
                    else:
                        nc.vector.tensor_copy(out=nx_store[:, t0:t1, 0:B], in_=src)
                t0 = t1

            # ---- recurrence ----
            nchunks = (T + XCHUNK - 1) // XCHUNK
            next_chunk = 0

            for s in range(S):
                h_prev = hist[:, s, :]          # [128, 2B] = [h1 | h2]
                h1_prev = hist[:, s, 0:B]
                h2_prev = hist[:, s, B:B2]

                h1_16 = hist16[:, s, 0:B]
                h2_16 = hist16[:, s, B:B2]
                # bank R: [r1 r2]; bank Z (negated): [-z1 -z2]; bank N: [n1h n2h]
                pr = ps.tile([128, B2], f32, tag="gr")
                nc.tensor.matmul(out=pr[:, 0:B], lhsT=Whh0_16(0), rhs=h1_16,
                                 start=True, stop=False)
                nc.tensor.matmul(out=pr[:, B:B2], lhsT=Whh1_16(0), rhs=h2_16,
                                 start=False, stop=True)
                pz = ps.tile([128, B2], f32, tag="gz")
                nc.tensor.matmul(out=pz[:, 0:B], lhsT=Whh0_16(1), rhs=h1_16,
                                 start=True, stop=False)
                nc.tensor.matmul(out=pz[:, B:B2], lhsT=Whh1_16(1), rhs=h2_16,
                                 start=False, stop=True)
                pn = ps.tile([128, B2], f32, tag="nh")
                nc.tensor.matmul(out=pn[:], lhsT=bhnmat16, rhs=selnh16,
                                 start=True, stop=False)
                nc.tensor.matmul(out=pn[:, 0:B], lhsT=Whh0_16(2), rhs=h1_16,
                                 start=False, stop=False)
                nc.tensor.matmul(out=pn[:, B:B2], lhsT=Whh1_16(2), rhs=h2_16,
                                 start=False, stop=True)

                pre_r = wp.tile([128, B2], f32, tag="prer")
                nc.vector.tensor_tensor(out=pre_r[:], in0=pr[:],
                                        in1=rz_store[:, s, 0:B2], op=Alu.add)
                sig_r = wp.tile([128, B2], f32, tag="sigr")
                nc.scalar.activation(sig_r[:], pre_r[:], Act.Sigmoid)
                pre_z = wp.tile([128, B2], f32, tag="prez")
                nc.vector.tensor_tensor(out=pre_z[:], in0=pz[:],
                                        in1=rz_store[:, s, B2:B4], op=Alu.add)
                # w = 1 - z = sigmoid(-pre_z); the z path is negated host-side
                w1z = wp.tile([128, B2], f32, tag="w1z")
                nc.scalar.activation(w1z[:], pre_z[:], Act.Sigmoid)
                tn = wp.tile([128, B2], f32, tag="tn")
                nc.vector.tensor_tensor(out=tn[:], in0=sig_r[:],
                                        in1=pn[:], op=Alu.mult)
                tn2 = wp.tile([128, B2], f32, tag="tn2")
                nc.vector.tensor_tensor(out=tn2[:], in0=tn[:],
                                        in1=nx_store[:, s, :], op=Alu.add)
                nt = wp.tile([128, B2], f32, tag="nt")
                nc.scalar.activation(nt[:], tn2[:], Act.Tanh)
                # v = z*h = h - w*h
                vwh = wp.tile([128, B2], f32, tag="vwh")
                nc.vector.tensor_tensor(out=vwh[:], in0=w1z[:],
                                        in1=h_prev, op=Alu.mult)
                vzh = wp.tile([128, B2], f32, tag="vzh")
                nc.vector.tensor_tensor(out=vzh[:], in0=h_prev,
                                        in1=vwh[:], op=Alu.subtract)
                uwn = wp.tile([128, B2], f32, tag="uwn")
                nc.vector.tensor_tensor(out=uwn[:], in0=w1z[:], in1=nt[:],
                                        op=Alu.mult)
                nc.vector.tensor_tensor(out=hist16[:, s + 1, :], in0=uwn[:],
                                        in1=vzh[:], op=Alu.add)
                nc.vector.tensor_tensor(out=hist[:, s + 1, :], in0=uwn[:],
                                        in1=vzh[:], op=Alu.add)

                # bulk xw2 chunk: after slot 8c+7 (h1 up to t'=8c+7 written)
                if next_chunk < nchunks and s == min(
                        XCHUNK * next_chunk + XCHUNK - 1, S - 1):
                    c = next_chunk
                    tp0 = XCHUNK * c
                    tp1 = min(tp0 + XCHUNK, T)
                    ccols = (tp1 - tp0) * B
                    px = psb.tile([128, 3, XCHUNK * B], f32, tag="bulk")
                    h1c = hist[:, tp0 + 1:tp1 + 1, 0:B]
                    for g in range(3):
                        nc.tensor.matmul(
                            out=px[:, g, 0:ccols], lhsT=B2g(g),
                            rhs=sel2[:, tp0 * B:tp0 * B + ccols],
                            start=True, stop=False)
                        nc.tensor.matmul(out=px[:, g, 0:ccols], lhsT=Wih1(g),
                                         rhs=h1c, start=False, stop=True)
                    so0 = tp0 + DELTA
                    so1 = tp1 + DELTA
                    for g, (store, col) in enumerate(
                            ((rz_store, B), (rz_store, 3 * B), (nx_store, B))):
                        nc.vector.tensor_copy(
                            out=store[:, so0:so1, col:col + B],
                            in_=px[:, g, 0:ccols].rearrange(
                                "p (t b) -> p t b", b=B))
                    next_chunk += 1

            # ---- scoring ----
            h2f = hist[:, S, B:B2]                     # final layer-2 states
            pbeta_t = psb.tile([1, 512], f32, tag="bulk")
            pbeta = pbeta_t[:, 0:B]
            nc.tensor.matmul(out=pbeta[:], lhsT=wh, rhs=h2f,
                             start=True, stop=True)
            beta = wp.tile([1, B], f32, tag="betasb")
            nc.vector.tensor_copy(out=beta[:], in_=pbeta[:])

            sc = pp.tile([1, S, B], f32, tag="sc")
            c0 = 0
            while c0 < SB:
                c1 = min(c0 + 512, SB)
                pscore = psb.tile([1, 512], f32, tag="bulk")
                nc.tensor.matmul(out=pscore[:, 0:c1 - c0], lhsT=ws_lo,
                                 rhs=x_sb[:, c0:c1], start=True, stop=False)
                nc.tensor.matmul(out=pscore[:, 0:c1 - c0], lhsT=ws_hi,
                                 rhs=x_sb[:, SB + c0:SB + c1],
                                 start=False, stop=True)
                nc.vector.tensor_copy(
                    out=sc[:].rearrange("p t b -> p (t b)")[:, c0:c1],
                    in_=pscore[:, 0:c1 - c0])
                c0 = c1

            # + beta[cluster], tanh(. + lin_b), mask, segment sums
            for b in range(B):
                nc.vector.tensor_scalar_add(out=sc[:, :, b:b + 1],
                                            in0=sc[:, :, b:b + 1],
                                            scalar1=beta[:, b:b + 1])
            th = pp.tile([1, S, B], f32, tag="th")
            nc.scalar.activation(th[:], sc[:], Act.Tanh, bias=lin_b)
            nc.vector.tensor_tensor(
                out=th[:], in0=th[:],
                in1=rmask.rearrange("p (t b) -> p t b", b=B), op=Alu.mult)
            sums = wp.tile([1, B], f32, tag="sums")
            for b in range(B):
                nc.vector.tensor_reduce(out=sums[:, b:b + 1],
                                        in_=th[:, :, b:b + 1],
                                        axis=mybir.AxisListType.XY, op=Alu.add)
            rsum = wp.tile([1, B], f32, tag="rsum")
            nc.vector.reciprocal(out=rsum[:], in_=sums[:])
            shalf = wp.tile([1, B], f32, tag="shalf")
            nc.vector.tensor_scalar_mul(out=shalf[:], in0=rsum[:],
                                        scalar1=P_SAL)
            fin = pp.tile([1, S, B], f32, tag="fin")
            for b in range(B):
                nc.vector.scalar_tensor_tensor(
                    out=fin[:, :, b:b + 1], in0=th[:, :, b:b + 1],
                    scalar=shalf[:, b:b + 1],
                    in1=pos_half.rearrange("p (t b) -> p t b", b=B)[:, :, b:b + 1],
                    op0=Alu.mult, op1=Alu.add)

            nc.sync.dma_start(out_dram.ap()[:],
                              fin[:].rearrange("p t b -> p (t b)"))

    nc.compile()
    return nc


def _prep_host(inputs):
    """Host-side sharding/packing.  Returns (T, S, in_maps, scatter)."""
    sent = np.ascontiguousarray(inputs["sent_gae_embeds"], dtype=np.float32)
    labels = np.asarray(inputs["labels"]).astype(np.int64)
    W_ih0 = np.asarray(inputs["W_ih0"], dtype=np.float32)
    W_hh0 = np.asarray(inputs["W_hh0"], dtype=np.float32)
    b_ih0 = np.asarray(inputs["b_ih0"], dtype=np.float32)
    b_hh0 = np.asarray(inputs["b_hh0"], dtype=np.float32)
    W_ih1 = np.asarray(inputs["W_ih1"], dtype=np.float32)
    W_hh1 = np.asarray(inputs["W_hh1"], dtype=np.float32)
    b_ih1 = np.asarray(inputs["b_ih1"], dtype=np.float32)
    b_hh1 = np.asarray(inputs["b_hh1"], dtype=np.float32)
    lin_v = np.asarray(inputs["lin_v"], dtype=np.float32)
    lin_g = np.asarray(inputs["lin_g"], dtype=np.float32)
    lin_b = np.asarray(inputs["lin_b"], dtype=np.float32)

    idx = [np.flatnonzero(labels == k) for k in range(K)]
    lens = np.array([max(len(i), 1) for i in idx])
    T = int(lens.max())
    S = T + DELTA
    SB = S * B

    # weight-normed linear
    Wn = (lin_g * lin_v / np.linalg.norm(lin_v, axis=1, keepdims=True))[0]
    w_s, w_h = Wn[:D], Wn[D:]

    # position scores (host constant table, permuted to cluster layout)
    inv = np.float32(1.0 / (N ** (1.0 / 3.0)))
    pos_full = np.maximum(np.float32(0.5),
                          np.exp(-(np.arange(N, dtype=np.float32) + 1.0) * inv))

    # shared weight pack
    WCOLS = 2 * 384 + 3 * 384 + 128 + 5
    gsign0 = np.ones(384, np.float32)
    gsign0[H:2 * H] = -1.0
    wpack = np.zeros((128, WCOLS), np.float32)
    wihT = W_ih0.T * gsign0[None, :]                 # [256, 384], z negated
    wpack[:, 0:384] = wihT[:128]
    wpack[:, 384:768] = wihT[128:]
    wpack[:, 768:1152] = W_hh0.T
    wpack[:, 1152:1536] = W_ih1.T * gsign0[None, :]
    wpack[:, 1536:1920] = W_hh1.T
    wpack[:, 1920:2048] = np.eye(128, dtype=np.float32)
    wpack[:, 2048] = w_s[:128]
    wpack[:, 2049] = w_s[128:]
    wpack[:, 2050] = w_h
    wpack[:, 2051] = b_hh0[2 * H:]
    wpack[:, 2052] = b_hh1[2 * H:]

    SCOLS = SB + T * B + 2 * B + 384 + 384 + 128
    o_sel2 = SB
    o_selnh = SB + T * B
    o_B1 = o_selnh + 2 * B
    o_B2 = o_B1 + 384
    o_bhn = o_B2 + 384

    spack_base = np.zeros((2, SCOLS), np.float32)
    # selnh: row0 -> first B cols (layer1), row1 -> second B cols.  (unused
    # now that bhn columns live in rz_store, but kept for layout stability)
    spack_base[0, o_selnh:o_selnh + B] = 1.0
    spack_base[1, o_selnh + B:o_selnh + 2 * B] = 1.0
    # B1: row0 = real bias (b_ih0 + b_hh0 for r,z; b_ih0 for n); row1 = pad
    breal = b_ih0.copy()
    breal[:2 * H] += b_hh0[:2 * H]
    breal[H:2 * H] *= -1.0
    spack_base[0, o_B1:o_B1 + 384] = breal
    spack_base[1, o_B1 + H:o_B1 + 2 * H] = -ZPAD
    breal2 = b_ih1.copy()
    breal2[:2 * H] += b_hh1[:2 * H]
    breal2[H:2 * H] *= -1.0
    spack_base[0, o_B2:o_B2 + 384] = breal2
    spack_base[1, o_B2 + H:o_B2 + 2 * H] = -ZPAD
    spack_base[0, o_bhn:o_bhn + 128] = b_hh0[2 * H:]
    spack_base[1, o_bhn:o_bhn + 128] = b_hh1[2 * H:]

    # The z-gate (gate 1) is negated everywhere so w = 1-z comes straight
    # out of sigmoid: sigma(-pre_z).
    gsign = np.ones(384, np.float32)
    gsign[H:2 * H] = -1.0
    w16pack = np.zeros((128, 768), np.float16)
    w16pack[:, 0:384] = (W_hh0.T * gsign[None, :]).astype(np.float16)
    w16pack[:, 384:768] = (W_hh1.T * gsign[None, :]).astype(np.float16)
    s16pack = np.zeros((2, 128 + 2 * B), np.float16)
    s16pack[0, 0:128] = b_hh0[2 * H:].astype(np.float16)
    s16pack[1, 0:128] = b_hh1[2 * H:].astype(np.float16)
    s16pack[0, 128:128 + B] = 1.0
    s16pack[1, 128 + B:128 + 2 * B] = 1.0

    in_maps = []
    scatter = []  # per core: list of (orig_index, col) pairs
    for d in range(NCORES):
        xp = np.zeros((128, 2 * SB), np.float32)
        sp = spack_base.copy()
        rp = np.zeros((1, 2 * SB + 1), np.float32)
        sc_pairs = []
        for b in range(B):
            k = d * B + b
            ids = idx[k]
            L = lens[k]
            pad = T - L
            # real slots t in [pad, pad+L); sentence j = t - pad
            cols = (np.arange(pad, pad + L) * B + b)
            if len(ids):
                xp[:, cols] = sent[ids, :128].T
                xp[:, SB + cols] = sent[ids, 128:].T
                rp[0, cols] = (1.0 - P_SAL) * pos_full[ids]
                sc_pairs.append((ids, cols))
            rp[0, SB + cols] = 1.0                       # mask
            sp[0, cols] = 1.0                            # sel1 real
            sp[1, cols] = 0.0
            pads1 = np.concatenate([np.arange(0, pad), np.arange(pad + L, S)])
            sp[1, pads1 * B + b] = 1.0
            # sel2 indexed by t' in [0, T)
            c2 = o_sel2 + np.arange(T) * B + b
            sp[0, c2[pad:pad + L]] = 1.0
            sp[1, c2[:pad]] = 1.0
            sp[1, c2[pad + L:]] = 1.0
        rp[0, 2 * SB] = lin_b[0]
        in_maps.append({"x_pack": xp, "w_pack": wpack, "s_pack": sp,
                        "r_pack": rp, "w16_pack": w16pack,
                        "s16_pack": s16pack})
        scatter.append(sc_pairs)

    return T, S, in_maps, scatter


_PROGRAM_CACHE = {}


def _install_ntff_hook_shim():
    """Provide antenv.axon_hooks (absent in this image) so that
    run_bass_kernel_spmd(trace=True) can capture NTFF profiles via the
    axon PJRT sidechannel.  Bench-only; never used by the grading path."""
    import contextlib
    import ctypes
    import types

    if "antenv.axon_hooks" in sys.modules:
        return
    so_path = "/opt/axon/libaxon_pjrt.so"
    hook = None
    if os.path.exists(so_path):
        lib = ctypes.CDLL(so_path)
        if hasattr(lib, "axon_start_nrt_profile"):
            lib.axon_start_nrt_profile.argtypes = [
                ctypes.POINTER(ctypes.c_int64), ctypes.c_size_t]
            lib.axon_start_nrt_profile.restype = ctypes.c_int64
            lib.axon_stop_nrt_profile.argtypes = [ctypes.c_char_p]
            lib.axon_stop_nrt_profile.restype = ctypes.c_int64

            @contextlib.contextmanager
            def _hook(output_dir, device_ids):
                import jax
                jax.devices()
                if device_ids:
                    ids = (ctypes.c_int64 * len(device_ids))(*device_ids)
                    rc = lib.axon_start_nrt_profile(ids, len(device_ids))
                else:
                    rc = lib.axon_start_nrt_profile(None, 0)
                if rc != 0:
                    raise RuntimeError(f"axon_start_nrt_profile rc={rc}")
                try:
                    yield
                finally:
                    n = lib.axon_stop_nrt_profile(str(output_dir).encode())
                    print(f"profile: {n} file(s) written to {output_dir}",
                          file=sys.stderr)

            hook = _hook

    mod = types.ModuleType("antenv.axon_hooks")
    mod.get_axon_ntff_profile_hook = lambda: hook
    mod.set_axon_ntff_profile_hook = lambda h: None
    sys.modules["antenv.axon_hooks"] = mod


def kernel(_bench=None, **inputs):
    from concourse import bass_utils

    if _bench is not None:
        _install_ntff_hook_shim()

    T, S, in_maps, scatter = _prep_host(inputs)

    key = (T, S)
    if key not in _PROGRAM_CACHE:
        _PROGRAM_CACHE[key] = _build_program(T, S)
    nc = _PROGRAM_CACHE[key]

    res = bass_utils.run_bass_kernel_spmd(
        nc, in_maps, core_ids=list(range(NCORES)),
        trace=_bench is not None, **(_bench or {}))

    out = np.zeros(N, np.float32)
    for d in range(NCORES):
        vals = res.results[d]["scores"][0]
        for ids, cols in scatter[d]:
            out[ids] = vals[cols]

    if _bench is not None:
        kernel._last_results = res
    return out


# revision 20
# speedup vs baseline: 2.8214x; 1.0306x over previous
"""Trainium2 Bass kernel for the cluster-GRU salience model.

Model (see reference): sentences are grouped by cluster label, each cluster's
sentence sequence is run through a 2-layer GRU, the final hidden state is
scattered back onto the cluster's sentences, scored through a weight-normed
linear + tanh, normalized by per-cluster segment sums, and mixed with a
positional score.

Strategy:
  - Host groups the N=4096 sentences by cluster (K=32), assigns 4 clusters to
    each of the 8 cores, and LEFT-pads every cluster sequence to the global
    max length T so all clusters finish at the same slot (uniform SPMD
    program).  Pad steps keep h frozen by forcing the update gate z to
    sigmoid(+30) ~= 1 via host-built selector matrices that feed a tiny
    bias matmul.
  - On device, both GRU layers run fused per slot (layer 2 lags DELTA slots),
    with gates on partitions ([H=128, B=4] tiles).  All xw + hw + bias adds
    are folded into PSUM matmul accumulation; the per-slot chain is
    sigmoid -> (r*hw_n) -> (+xw_n) -> tanh -> (1-z)*n -> +z*h.
  - Layer-2's input transform (W_ih1 @ h1) is computed in bulk every 8 slots.
  - Scoring reuses the on-chip transposed embeddings: a [1 x cols] matvec,
    per-cluster beta add, tanh, masked per-cluster segment sums, and a fused
    (score * 0.5/sum + pos/2) epilogue.
"""

import os
import sys

import numpy as np

for _p in ("/opt/trn_rl_repo",):
    if _p not in sys.path and os.path.isdir(_p):
        sys.path.insert(0, _p)

N = 4096
K = 32
D = 256
H = 128
P_SAL = 0.5
NCORES = 8
B = K // NCORES          # clusters per core
DELTA = 20               # layer-2 lag (slots)
XCHUNK = 16              # slots per bulk xw2 chunk
ZPAD = 30.0              # pad-step z-gate pre-activation (sigmoid(30) ~ 1)


def _build_program(T, S):
    """Build + compile the (shared, SPMD) Bass program.

    T: global max cluster length; S = T + DELTA total recurrence slots.
    All per-core variation lives in the input data, not the program.
    """
    import concourse.bacc as bacc
    import concourse.mybir as mybir
    import concourse.tile as tile

    f32 = mybir.dt.float32
    Alu = mybir.AluOpType
    Act = mybir.ActivationFunctionType

    SB = S * B
    B2, B4, B6 = 2 * B, 4 * B, 6 * B

    nc = bacc.Bacc("TRN2", target_bir_lowering=False, debug=False,
                   num_devices=NCORES)

    # ---- DRAM tensors (per-core inputs) ----
    # x_pack: transposed, cluster-major padded embeddings.
    #   [:, 0:SB]      = features   0:128 at col t*B+b
    #   [:, SB:2*SB]   = features 128:256 at col t*B+b
    x_dram = nc.dram_tensor("x_pack", [128, 2 * SB], f32, kind="ExternalInput")
    # wpack columns: Wih0T lo,hi (2*384) | Whh0T (384) | Wih1T (384) |
    #                Whh1T (384) | ident (128) | ws lo,hi (2) | wh (1) | bhn0|bhn1 (2)
    WCOLS = 2 * 384 + 3 * 384 + 128 + 5
    w_dram = nc.dram_tensor("w_pack", [128, WCOLS], f32, kind="ExternalInput")
    # spack: [2, x] selector/bias rows:
    #   sel1 (SB) | sel2 (T*B) | selnh (2B) | B1 (384) | B2 (384) | bhn (128)
    SCOLS = SB + T * B + B2 + 384 + 384 + 128
    s_dram = nc.dram_tensor("s_pack", [2, SCOLS], f32, kind="ExternalInput")
    # rpack: [1, x]: pos_half (SB) | mask (SB) | lin_b (1)
    r_dram = nc.dram_tensor("r_pack", [1, 2 * SB + 1], f32, kind="ExternalInput")
    f16 = mybir.dt.float16
    # fp16 packs for the per-slot gate matmuls
    w16_dram = nc.dram_tensor("w16_pack", [128, 768], f16, kind="ExternalInput")
    s16_dram = nc.dram_tensor("s16_pack", [2, 128 + B2], f16, kind="ExternalInput")

    out_dram = nc.dram_tensor("scores", [1, SB], f32, kind="ExternalOutput")

    with tile.TileContext(nc) as tc:
        with (
            tc.tile_pool(name="persist", bufs=1) as pp,
            tc.tile_pool(name="work", bufs=3) as wp,
            tc.tile_pool(name="ps", bufs=2, space="PSUM") as ps,
            tc.tile_pool(name="psn", bufs=1, space="PSUM") as psn,
            tc.tile_pool(name="psbulk", bufs=2, space="PSUM") as psb,
        ):
            # ---- load inputs into SBUF ----
            x_sb = pp.tile([128, 2 * SB], f32, tag="x")
            w_sb = pp.tile([128, WCOLS], f32, tag="w")
            s_sb = pp.tile([2, SCOLS], f32, tag="s")
            r_sb = pp.tile([1, 2 * SB + 1], f32, tag="r")
            nc.sync.dma_start(x_sb[:], x_dram.ap()[:])
            nc.sync.dma_start(w_sb[:], w_dram.ap()[:])
            nc.sync.dma_start(s_sb[:], s_dram.ap()[:])
            nc.sync.dma_start(r_sb[:], r_dram.ap()[:])
            w16_sb = pp.tile([128, 768], f16, tag="w16")
            s16_sb = pp.tile([2, 128 + B2], f16, tag="s16")
            nc.scalar.dma_start(w16_sb[:], w16_dram.ap()[:])
            nc.scalar.dma_start(s16_sb[:], s16_dram.ap()[:])
            Whh0_16 = lambda g: w16_sb[:, g * 128:(g + 1) * 128]
            Whh1_16 = lambda g: w16_sb[:, 384 + g * 128:384 + (g + 1) * 128]
            bhnmat16 = s16_sb[:, 0:128]
            selnh16 = s16_sb[:, 128:128 + B2]

            # named views of the packs
            Wih0_lo = lambda g: w_sb[:, g * 128:(g + 1) * 128]
            Wih0_hi = lambda g: w_sb[:, 384 + g * 128:384 + (g + 1) * 128]
            Whh0 = lambda g: w_sb[:, 768 + g * 128:768 + (g + 1) * 128]
            Wih1 = lambda g: w_sb[:, 1152 + g * 128:1152 + (g + 1) * 128]
            Whh1 = lambda g: w_sb[:, 1536 + g * 128:1536 + (g + 1) * 128]
            ident = w_sb[:, 1920:2048]
            ws_lo = w_sb[:, 2048:2049]
            ws_hi = w_sb[:, 2049:2050]
            wh = w_sb[:, 2050:2051]
            bhn0 = w_sb[:, 2051:2052]
            bhn1 = w_sb[:, 2052:2053]

            o_sel2 = SB
            o_selnh = SB + T * B
            o_B1 = o_selnh + B2
            o_B2 = o_B1 + 384
            o_bhn = o_B2 + 384
            sel1 = s_sb[:, 0:SB]
            sel2 = s_sb[:, o_sel2:o_sel2 + T * B]
            selnh = s_sb[:, o_selnh:o_selnh + B2]
            B1g = lambda g: s_sb[:, o_B1 + g * 128:o_B1 + (g + 1) * 128]
            B2g = lambda g: s_sb[:, o_B2 + g * 128:o_B2 + (g + 1) * 128]
            bhnmat = s_sb[:, o_bhn:o_bhn + 128]

            pos_half = r_sb[:, 0:SB]
            rmask = r_sb[:, SB:2 * SB]
            lin_b = r_sb[:, 2 * SB:2 * SB + 1]

            # ---- persistent state / stores ----
            # rz_store[:, s, :]: a1_r | xw2_r | a1_z | xw2_z
            rz_store = pp.tile([128, S, B4], f32, tag="rzs")
            # nx_store[:, s, :]: a1_n | xw2_n
            nx_store = pp.tile([128, S, B2], f32, tag="nxs")
            # hist[:, s, :]: [h1 | h2] state BEFORE slot s (fp32 carry);
            # hist16 is the fp16 shadow feeding the PE gate matmuls
            hist = pp.tile([128, S + 1, B2], f32, tag="hist")
            hist16 = pp.tile([128, S + 1, B2], f16, tag="hist16")
            nc.vector.memset(hist[:, 0:1, :], 0.0)
            nc.vector.memset(hist16[:, 0:1, :], 0.0)
            # layer-2 parts of store slots [0, DELTA): force z2 pad
            nc.vector.memset(rz_store[:, 0:DELTA, B:B2], 0.0)
            nc.vector.memset(rz_store[:, 0:DELTA, 3 * B:B4], -ZPAD)
            nc.vector.memset(nx_store[:, 0:DELTA, B:B2], 0.0)

            # ---- phase A: a1 = W_ih0 @ x (+ biases via selector MM) ----
            # chunks of 128 slots (512 cols)
            t0 = 0
            while t0 < S:
                t1 = min(t0 + 128, S)
                cols = (t1 - t0) * B
                c0 = t0 * B
                for g in range(3):
                    pa = psb.tile([128, 512], f32, tag="bulk")
                    nc.tensor.matmul(out=pa[:, 0:cols], lhsT=B1g(g),
                                     rhs=sel1[:, c0:c0 + cols],
                                     start=True, stop=False)
                    nc.tensor.matmul(out=pa[:, 0:cols], lhsT=Wih0_lo(g),
                                     rhs=x_sb[:, c0:c0 + cols],
                                     start=False, stop=False)
                    nc.tensor.matmul(out=pa[:, 0:cols], lhsT=Wih0_hi(g),
                                     rhs=x_sb[:, SB + c0:SB + c0 + cols],
                                     start=False, stop=True)
                    src = pa[:, 0:cols].rearrange("p (t b) -> p t b", b=B)
                    if g == 0:
                        nc.vector.tensor_copy(out=rz_store[:, t0:t1, 0:B], in_=src)
                    elif g == 1:
                        nc.vector.tensor_copy(out=rz_store[:, t0:t1, 2 * B:3 * B],
                                              in_=src)
                    else:
                        nc.vector.tensor_copy(out=nx_store[:, t0:t1, 0:B], in_=src)
                t0 = t1

            # ---- recurrence ----
            nchunks = (T + XCHUNK - 1) // XCHUNK
            next_chunk = 0

            for s in range(S):
                h_prev = hist[:, s, :]          # [128, 2B] = [h1 | h2]
                h1_prev = hist[:, s, 0:B]
                h2_prev = hist[:, s, B:B2]

                h1_16 = hist16[:, s, 0:B]
                h2_16 = hist16[:, s, B:B2]
                # bank R: [r1 r2]; bank Z (negated): [-z1 -z2]; bank N: [n1h n2h]
                pr = ps.tile([128, B2], f32, tag="gr")
                nc.tensor.matmul(out=pr[:, 0:B], lhsT=Whh0_16(0), rhs=h1_16,
                                 start=True, stop=False)
                nc.tensor.matmul(out=pr[:, B:B2], lhsT=Whh1_16(0), rhs=h2_16,
                                 start=False, stop=True)
                pz = ps.tile([128, B2], f32, tag="gz")
                nc.tensor.matmul(out=pz[:, 0:B], lhsT=Whh0_16(1), rhs=h1_16,
                                 start=True, stop=False)
                nc.tensor.matmul(out=pz[:, B:B2], lhsT=Whh1_16(1), rhs=h2_16,
                                 start=False, stop=True)
                pn = psn.tile([128, B2], f32, tag="nh")
                nc.tensor.matmul(out=pn[:], lhsT=bhnmat16, rhs=selnh16,
                                 start=True, stop=False)
                nc.tensor.matmul(out=pn[:, 0:B], lhsT=Whh0_16(2), rhs=h1_16,
                                 start=False, stop=False)
                nc.tensor.matmul(out=pn[:, B:B2], lhsT=Whh1_16(2), rhs=h2_16,
                                 start=False, stop=True)

                # pre-activations added in place in PSUM (sigmoid reads PSUM)
                nc.vector.tensor_tensor(out=pr[:], in0=pr[:],
                                        in1=rz_store[:, s, 0:B2], op=Alu.add)
                sig_r = wp.tile([128, B2], f32, tag="sigr")
                nc.scalar.activation(sig_r[:], pr[:], Act.Sigmoid)
                nc.vector.tensor_tensor(out=pz[:], in0=pz[:],
                                        in1=rz_store[:, s, B2:B4], op=Alu.add)
                # w = 1 - z = sigmoid(-pre_z); the z path is negated host-side
                w1z = wp.tile([128, B2], f32, tag="w1z")
                nc.scalar.activation(w1z[:], pz[:], Act.Sigmoid)
                tn = wp.tile([128, B2], f32, tag="tn")
                nc.vector.tensor_tensor(out=tn[:], in0=sig_r[:],
                                        in1=pn[:], op=Alu.mult)
                tn2 = psn.tile([128, B2], f32, tag="tn2")
                nc.vector.tensor_tensor(out=tn2[:], in0=tn[:],
                                        in1=nx_store[:, s, :], op=Alu.add)
                nt = wp.tile([128, B2], f32, tag="nt")
                nc.scalar.activation(nt[:], tn2[:], Act.Tanh)
                # v = z*h = h - w*h (off the critical chain; emitted after tn2)
                vwh = wp.tile([128, B2], f32, tag="vwh")
                nc.vector.tensor_tensor(out=vwh[:], in0=w1z[:],
                                        in1=h_prev, op=Alu.mult)
                vzh = wp.tile([128, B2], f32, tag="vzh")
                nc.vector.tensor_tensor(out=vzh[:], in0=h_prev,
                                        in1=vwh[:], op=Alu.subtract)
                uwn = wp.tile([128, B2], f32, tag="uwn")
                nc.vector.tensor_tensor(out=uwn[:], in0=w1z[:], in1=nt[:],
                                        op=Alu.mult)
                nc.vector.tensor_tensor(out=hist16[:, s + 1, :], in0=uwn[:],
                                        in1=vzh[:], op=Alu.add)
                nc.vector.tensor_tensor(out=hist[:, s + 1, :], in0=uwn[:],
                                        in1=vzh[:], op=Alu.add)

                # bulk xw2 chunk: after slot 8c+7 (h1 up to t'=8c+7 written)
                if next_chunk < nchunks and s == min(
                        XCHUNK * next_chunk + XCHUNK - 1, S - 1):
                    c = next_chunk
                    tp0 = XCHUNK * c
                    tp1 = min(tp0 + XCHUNK, T)
                    ccols = (tp1 - tp0) * B
                    px = psb.tile([128, 3, XCHUNK * B], f32, tag="bulk")
                    h1c = hist[:, tp0 + 1:tp1 + 1, 0:B]
                    for g in range(3):
                        nc.tensor.matmul(
                            out=px[:, g, 0:ccols], lhsT=B2g(g),
                            rhs=sel2[:, tp0 * B:tp0 * B + ccols],
                            start=True, stop=False)
                        nc.tensor.matmul(out=px[:, g, 0:ccols], lhsT=Wih1(g),
                                         rhs=h1c, start=False, stop=True)
                    so0 = tp0 + DELTA
                    so1 = tp1 + DELTA
                    for g, (store, col) in enumerate(
                            ((rz_store, B), (rz_store, 3 * B), (nx_store, B))):
                        nc.vector.tensor_copy(
                            out=store[:, so0:so1, col:col + B],
                            in_=px[:, g, 0:ccols].rearrange(
                                "p (t b) -> p t b", b=B))
                    next_chunk += 1

            # ---- scoring ----
            h2f = hist[:, S, B:B2]                     # final layer-2 states
            pbeta_t = psb.tile([1, 512], f32, tag="bulk")
            pbeta = pbeta_t[:, 0:B]
            nc.tensor.matmul(out=pbeta[:], lhsT=wh, rhs=h2f,
                             start=True, stop=True)
            beta = wp.tile([1, B], f32, tag="betasb")
            nc.vector.tensor_copy(out=beta[:], in_=pbeta[:])

            sc = pp.tile([1, S, B], f32, tag="sc")
            c0 = 0
            while c0 < SB:
                c1 = min(c0 + 512, SB)
                pscore = psb.tile([1, 512], f32, tag="bulk")
                nc.tensor.matmul(out=pscore[:, 0:c1 - c0], lhsT=ws_lo,
                                 rhs=x_sb[:, c0:c1], start=True, stop=False)
                nc.tensor.matmul(out=pscore[:, 0:c1 - c0], lhsT=ws_hi,
                                 rhs=x_sb[:, SB + c0:SB + c1],
                                 start=False, stop=True)
                nc.vector.tensor_copy(
                    out=sc[:].rearrange("p t b -> p (t b)")[:, c0:c1],
                    in_=pscore[:, 0:c1 - c0])
                c0 = c1

            # + beta[cluster], tanh(. + lin_b), mask, segment sums
            for b in range(B):
                nc.vector.tensor_scalar_add(out=sc[:, :, b:b + 1],
                                            in0=sc[:, :, b:b + 1],
                                            scalar1=beta[:, b:b + 1])
            th = pp.tile([1, S, B], f32, tag="th")
            nc.scalar.activation(th[:], sc[:], Act.Tanh, bias=lin_b)
            nc.vector.tensor_tensor(
                out=th[:], in0=th[:],
                in1=rmask.rearrange("p (t b) -> p t b", b=B), op=Alu.mult)
            sums = wp.tile([1, B], f32, tag="sums")
            for b in range(B):
                nc.vector.tensor_reduce(out=sums[:, b:b + 1],
                                        in_=th[:, :, b:b + 1],
                                        axis=mybir.AxisListType.XY, op=Alu.add)
            rsum = wp.tile([1, B], f32, tag="rsum")
            nc.vector.reciprocal(out=rsum[:], in_=sums[:])
            shalf = wp.tile([1, B], f32, tag="shalf")
            nc.vector.tensor_scalar_mul(out=shalf[:], in0=rsum[:],
                                        scalar1=P_SAL)
            fin = pp.tile([1, S, B], f32, tag="fin")
            for b in range(B):
                nc.vector.scalar_tensor_tensor(
                    out=fin[:, :, b:b + 1], in0=th[:, :, b:b + 1],
                    scalar=shalf[:, b:b + 1],
                    in1=pos_half.rearrange("p (t b) -> p t b", b=B)[:, :, b:b + 1],
                    op0=Alu.mult, op1=Alu.add)

            nc.sync.dma_start(out_dram.ap()[:],
                              fin[:].rearrange("p t b -> p (t b)"))

    nc.compile()
    return nc


def _prep_host(inputs):
    """Host-side sharding/packing.  Returns (T, S, in_maps, scatter)."""
    sent = np.ascontiguousarray(inputs["sent_gae_embeds"], dtype=np.float32)
    labels = np.asarray(inputs["labels"]).astype(np.int64)
    W_ih0 = np.asarray(inputs["W_ih0"], dtype=np.float32)
    W_hh0 = np.asarray(inputs["W_hh0"], dtype=np.float32)
    b_ih0 = np.asarray(inputs["b_ih0"], dtype=np.float32)
    b_hh0 = np.asarray(inputs["b_hh0"], dtype=np.float32)
    W_ih1 = np.asarray(inputs["W_ih1"], dtype=np.float32)
    W_hh1 = np.asarray(inputs["W_hh1"], dtype=np.float32)
    b_ih1 = np.asarray(inputs["b_ih1"], dtype=np.float32)
    b_hh1 = np.asarray(inputs["b_hh1"], dtype=np.float32)
    lin_v = np.asarray(inputs["lin_v"], dtype=np.float32)
    lin_g = np.asarray(inputs["lin_g"], dtype=np.float32)
    lin_b = np.asarray(inputs["lin_b"], dtype=np.float32)

    idx = [np.flatnonzero(labels == k) for k in range(K)]
    lens = np.array([max(len(i), 1) for i in idx])
    T = int(lens.max())
    S = T + DELTA
    SB = S * B

    # weight-normed linear
    Wn = (lin_g * lin_v / np.linalg.norm(lin_v, axis=1, keepdims=True))[0]
    w_s, w_h = Wn[:D], Wn[D:]

    # position scores (host constant table, permuted to cluster layout)
    inv = np.float32(1.0 / (N ** (1.0 / 3.0)))
    pos_full = np.maximum(np.float32(0.5),
                          np.exp(-(np.arange(N, dtype=np.float32) + 1.0) * inv))

    # shared weight pack
    WCOLS = 2 * 384 + 3 * 384 + 128 + 5
    gsign0 = np.ones(384, np.float32)
    gsign0[H:2 * H] = -1.0
    wpack = np.zeros((128, WCOLS), np.float32)
    wihT = W_ih0.T * gsign0[None, :]                 # [256, 384], z negated
    wpack[:, 0:384] = wihT[:128]
    wpack[:, 384:768] = wihT[128:]
    wpack[:, 768:1152] = W_hh0.T
    wpack[:, 1152:1536] = W_ih1.T * gsign0[None, :]
    wpack[:, 1536:1920] = W_hh1.T
    wpack[:, 1920:2048] = np.eye(128, dtype=np.float32)
    wpack[:, 2048] = w_s[:128]
    wpack[:, 2049] = w_s[128:]
    wpack[:, 2050] = w_h
    wpack[:, 2051] = b_hh0[2 * H:]
    wpack[:, 2052] = b_hh1[2 * H:]

    SCOLS = SB + T * B + 2 * B + 384 + 384 + 128
    o_sel2 = SB
    o_selnh = SB + T * B
    o_B1 = o_selnh + 2 * B
    o_B2 = o_B1 + 384
    o_bhn = o_B2 + 384

    spack_base = np.zeros((2, SCOLS), np.float32)
    # selnh: row0 -> first B cols (layer1), row1 -> second B cols.  (unused
    # now that bhn columns live in rz_store, but kept for layout stability)
    spack_base[0, o_selnh:o_selnh + B] = 1.0
    spack_base[1, o_selnh + B:o_selnh + 2 * B] = 1.0
    # B1: row0 = real bias (b_ih0 + b_hh0 for r,z; b_ih0 for n); row1 = pad
    breal = b_ih0.copy()
    breal[:2 * H] += b_hh0[:2 * H]
    breal[H:2 * H] *= -1.0
    spack_base[0, o_B1:o_B1 + 384] = breal
    spack_base[1, o_B1 + H:o_B1 + 2 * H] = -ZPAD
    breal2 = b_ih1.copy()
    breal2[:2 * H] += b_hh1[:2 * H]
    breal2[H:2 * H] *= -1.0
    spack_base[0, o_B2:o_B2 + 384] = breal2
    spack_base[1, o_B2 + H:o_B2 + 2 * H] = -ZPAD
    spack_base[0, o_bhn:o_bhn + 128] = b_hh0[2 * H:]
    spack_base[1, o_bhn:o_bhn + 128] = b_hh1[2 * H:]

    # The z-gate (gate 1) is negated everywhere so w = 1-z comes straight
    # out of sigmoid: sigma(-pre_z).
    gsign = np.ones(384, np.float32)
    gsign[H:2 * H] = -1.0
    w16pack = np.zeros((128, 768), np.float16)
    w16pack[:, 0:384] = (W_hh0.T * gsign[None, :]).astype(np.float16)
    w16pack[:, 384:768] = (W_hh1.T * gsign[None, :]).astype(np.float16)
    s16pack = np.zeros((2, 128 + 2 * B), np.float16)
    s16pack[0, 0:128] = b_hh0[2 * H:].astype(np.float16)
    s16pack[1, 0:128] = b_hh1[2 * H:].astype(np.float16)
    s16pack[0, 128:128 + B] = 1.0
    s16pack[1, 128 + B:128 + 2 * B] = 1.0

    in_maps = []
    scatter = []  # per core: list of (orig_index, col) pairs
    for d in range(NCORES):
        xp = np.zeros((128, 2 * SB), np.float32)
        sp = spack_base.copy()
        rp = np.zeros((1, 2 * SB + 1), np.float32)
        sc_pairs = []
        for b in range(B):
            k = d * B + b
            ids = idx[k]
            L = lens[k]
            pad = T - L
            # real slots t in [pad, pad+L); sentence j = t - pad
            cols = (np.arange(pad, pad + L) * B + b)
            if len(ids):
                xp[:, cols] = sent[ids, :128].T
                xp[:, SB + cols] = sent[ids, 128:].T
                rp[0, cols] = (1.0 - P_SAL) * pos_full[ids]
                sc_pairs.append((ids, cols))
            rp[0, SB + cols] = 1.0                       # mask
            sp[0, cols] = 1.0                            # sel1 real
            sp[1, cols] = 0.0
            pads1 = np.concatenate([np.arange(0, pad), np.arange(pad + L, S)])
            sp[1, pads1 * B + b] = 1.0
            # sel2 indexed by t' in [0, T)
            c2 = o_sel2 + np.arange(T) * B + b
            sp[0, c2[pad:pad + L]] = 1.0
            sp[1, c2[:pad]] = 1.0
            sp[1, c2[pad + L:]] = 1.0
        rp[0, 2 * SB] = lin_b[0]
        in_maps.append({"x_pack": xp, "w_pack": wpack, "s_pack": sp,
                        "r_pack": rp, "w16_pack": w16pack,
                        "s16_pack": s16pack})
        scatter.append(sc_pairs)

    return T, S, in_maps, scatter


_PROGRAM_CACHE = {}


def _install_ntff_hook_shim():
    """Provide antenv.axon_hooks (absent in this image) so that
    run_bass_kernel_spmd(trace=True) can capture NTFF profiles via the
    axon PJRT sidechannel.  Bench-only; never used by the grading path."""
    import contextlib
    import ctypes
    import types

    if "antenv.axon_hooks" in sys.modules:
        return
    so_path = "/opt/axon/libaxon_pjrt.so"
    hook = None
    if os.path.exists(so_path):
        lib = ctypes.CDLL(so_path)
        if hasattr(lib, "axon_start_nrt_profile"):
            lib.axon_start_nrt_profile.argtypes = [
                ctypes.POINTER(ctypes.c_int64), ctypes.c_size_t]
            lib.axon_start_nrt_profile.restype = ctypes.c_int64
            lib.axon_stop_nrt_profile.argtypes = [ctypes.c_char_p]
            lib.axon_stop_nrt_profile.restype = ctypes.c_int64

            @contextlib.contextmanager
            def _hook(output_dir, device_ids):
                import jax
                jax.devices()
                if device_ids:
                    ids = (ctypes.c_int64 * len(device_ids))(*device_ids)
                    rc = lib.axon_start_nrt_profile(ids, len(device_ids))
                else:
                    rc = lib.axon_start_nrt_profile(None, 0)
                if rc != 0:
                    raise RuntimeError(f"axon_start_nrt_profile rc={rc}")
                try:
                    yield
                finally:
                    n = lib.axon_stop_nrt_profile(str(output_dir).encode())
                    print(f"profile: {n} file(s) written to {output_dir}",
                          file=sys.stderr)

            hook = _hook

    mod = types.ModuleType("antenv.axon_hooks")
    mod.get_axon_ntff_profile_hook = lambda: hook
    mod.set_axon_ntff_profile_hook = lambda h: None
    sys.modules["antenv.axon_hooks"] = mod


def kernel(_bench=None, **inputs):
    from concourse import bass_utils

    if _bench is not None:
        _install_ntff_hook_shim()

    T, S, in_maps, scatter = _prep_host(inputs)

    key = (T, S)
    if key not in _PROGRAM_CACHE:
        _PROGRAM_CACHE[key] = _build_program(T, S)
    nc = _PROGRAM_CACHE[key]

    res = bass_utils.run_bass_kernel_spmd(
        nc, in_maps, core_ids=list(range(NCORES)),
        trace=_bench is not None, **(_bench or {}))

    out = np.zeros(N, np.float32)
    for d in range(NCORES):
        vals = res.results[d]["scores"][0]
        for ids, cols in scatter[d]:
            out[ids] = vals[cols]

    if _bench is not None:
        kernel._last_results = res
    return out


# revision 22
# speedup vs baseline: 2.8936x; 1.0256x over previous
"""Trainium2 Bass kernel for the cluster-GRU salience model.

Model (see reference): sentences are grouped by cluster label, each cluster's
sentence sequence is run through a 2-layer GRU, the final hidden state is
scattered back onto the cluster's sentences, scored through a weight-normed
linear + tanh, normalized by per-cluster segment sums, and mixed with a
positional score.

Strategy:
  - Host groups the N=4096 sentences by cluster (K=32), assigns 4 clusters to
    each of the 8 cores, and LEFT-pads every cluster sequence to the global
    max length T so all clusters finish at the same slot (uniform SPMD
    program).  Pad steps keep h frozen by forcing the update gate z to
    sigmoid(+30) ~= 1 via host-built selector matrices that feed a tiny
    bias matmul.
  - On device, both GRU layers run fused per slot (layer 2 lags DELTA slots),
    with gates on partitions ([H=128, B=4] tiles).  All xw + hw + bias adds
    are folded into PSUM matmul accumulation; the per-slot chain is
    sigmoid -> (r*hw_n) -> (+xw_n) -> tanh -> (1-z)*n -> +z*h.
  - Layer-2's input transform (W_ih1 @ h1) is computed in bulk every 8 slots.
  - Scoring reuses the on-chip transposed embeddings: a [1 x cols] matvec,
    per-cluster beta add, tanh, masked per-cluster segment sums, and a fused
    (score * 0.5/sum + pos/2) epilogue.
"""

import os
import sys

import numpy as np

for _p in ("/opt/trn_rl_repo",):
    if _p not in sys.path and os.path.isdir(_p):
        sys.path.insert(0, _p)

N = 4096
K = 32
D = 256
H = 128
P_SAL = 0.5
NCORES = 8
B = K // NCORES          # clusters per core
DELTA = 20               # layer-2 lag (slots)
XCHUNK = 16              # slots per bulk xw2 chunk
ZPAD = 30.0              # pad-step z-gate pre-activation (sigmoid(30) ~ 1)


def _build_program(T, S):
    """Build + compile the (shared, SPMD) Bass program.

    T: global max cluster length; S = T + DELTA total recurrence slots.
    All per-core variation lives in the input data, not the program.
    """
    import concourse.bacc as bacc
    import concourse.mybir as mybir
    import concourse.tile as tile

    f32 = mybir.dt.float32
    Alu = mybir.AluOpType
    Act = mybir.ActivationFunctionType

    SB = S * B
    B2, B4, B6 = 2 * B, 4 * B, 6 * B

    nc = bacc.Bacc("TRN2", target_bir_lowering=False, debug=False,
                   num_devices=NCORES)

    # ---- DRAM tensors (per-core inputs) ----
    # x_pack: transposed, cluster-major padded embeddings.
    #   [:, 0:SB]      = features   0:128 at col t*B+b
    #   [:, SB:2*SB]   = features 128:256 at col t*B+b
    x_dram = nc.dram_tensor("x_pack", [128, 2 * SB], f32, kind="ExternalInput")
    # wpack columns: Wih0T lo,hi (2*384) | Whh0T (384) | Wih1T (384) |
    #                Whh1T (384) | ident (128) | ws lo,hi (2) | wh (1) | bhn0|bhn1 (2)
    WCOLS = 2 * 384 + 3 * 384 + 128 + 5
    w_dram = nc.dram_tensor("w_pack", [128, WCOLS], f32, kind="ExternalInput")
    # spack: [2, x] selector/bias rows:
    #   sel1 (SB) | sel2 (T*B) | selnh (2B) | B1 (384) | B2 (384) | bhn (128)
    SCOLS = SB + T * B + B2 + 384 + 384 + 128
    s_dram = nc.dram_tensor("s_pack", [2, SCOLS], f32, kind="ExternalInput")
    # rpack: [1, x]: pos_half (SB) | mask (SB) | lin_b (1)
    r_dram = nc.dram_tensor("r_pack", [1, 2 * SB + 1], f32, kind="ExternalInput")
    f16 = mybir.dt.float16
    # fp16 packs for the per-slot gate matmuls
    w16_dram = nc.dram_tensor("w16_pack", [128, 1536], f16, kind="ExternalInput")
    s16_dram = nc.dram_tensor("s16_pack", [2, 128 + B2 + 384 + T * B], f16,
                              kind="ExternalInput")

    out_dram = nc.dram_tensor("scores", [1, SB], f32, kind="ExternalOutput")

    with tile.TileContext(nc) as tc:
        with (
            tc.tile_pool(name="persist", bufs=1) as pp,
            tc.tile_pool(name="work", bufs=3) as wp,
            tc.tile_pool(name="ps", bufs=2, space="PSUM") as ps,
            tc.tile_pool(name="psn", bufs=1, space="PSUM") as psn,
            tc.tile_pool(name="psbulk", bufs=2, space="PSUM") as psb,
        ):
            # ---- load inputs into SBUF ----
            x_sb = pp.tile([128, 2 * SB], f32, tag="x")
            w_sb = pp.tile([128, WCOLS], f32, tag="w")
            s_sb = pp.tile([2, SCOLS], f32, tag="s")
            r_sb = pp.tile([1, 2 * SB + 1], f32, tag="r")
            nc.sync.dma_start(x_sb[:], x_dram.ap()[:])
            nc.sync.dma_start(w_sb[:], w_dram.ap()[:])
            nc.sync.dma_start(s_sb[:], s_dram.ap()[:])
            nc.sync.dma_start(r_sb[:], r_dram.ap()[:])
            w16_sb = pp.tile([128, 1536], f16, tag="w16")
            s16_sb = pp.tile([2, 128 + B2 + 384 + T * B], f16, tag="s16")
            nc.scalar.dma_start(w16_sb[:], w16_dram.ap()[:])
            nc.scalar.dma_start(s16_sb[:], s16_dram.ap()[:])
            Whh0_16 = lambda g: w16_sb[:, g * 128:(g + 1) * 128]
            Whh1_16 = lambda g: w16_sb[:, 384 + g * 128:384 + (g + 1) * 128]
            Wih1_hi16 = lambda g: w16_sb[:, 768 + g * 128:768 + (g + 1) * 128]
            Wih1_lo16 = lambda g: w16_sb[:, 1152 + g * 128:1152 + (g + 1) * 128]
            bhnmat16 = s16_sb[:, 0:128]
            selnh16 = s16_sb[:, 128:128 + B2]
            B2g16 = lambda g: s16_sb[:, 128 + B2 + g * 128:128 + B2 + (g + 1) * 128]
            o_s2_16 = 128 + B2 + 384
            sel2_16 = s16_sb[:, o_s2_16:o_s2_16 + T * B]

            # named views of the packs
            Wih0_lo = lambda g: w_sb[:, g * 128:(g + 1) * 128]
            Wih0_hi = lambda g: w_sb[:, 384 + g * 128:384 + (g + 1) * 128]
            Whh0 = lambda g: w_sb[:, 768 + g * 128:768 + (g + 1) * 128]
            Wih1 = lambda g: w_sb[:, 1152 + g * 128:1152 + (g + 1) * 128]
            Whh1 = lambda g: w_sb[:, 1536 + g * 128:1536 + (g + 1) * 128]
            ident = w_sb[:, 1920:2048]
            ws_lo = w_sb[:, 2048:2049]
            ws_hi = w_sb[:, 2049:2050]
            wh = w_sb[:, 2050:2051]
            bhn0 = w_sb[:, 2051:2052]
            bhn1 = w_sb[:, 2052:2053]

            o_sel2 = SB
            o_selnh = SB + T * B
            o_B1 = o_selnh + B2
            o_B2 = o_B1 + 384
            o_bhn = o_B2 + 384
            sel1 = s_sb[:, 0:SB]
            sel2 = s_sb[:, o_sel2:o_sel2 + T * B]
            selnh = s_sb[:, o_selnh:o_selnh + B2]
            B1g = lambda g: s_sb[:, o_B1 + g * 128:o_B1 + (g + 1) * 128]
            B2g = lambda g: s_sb[:, o_B2 + g * 128:o_B2 + (g + 1) * 128]
            bhnmat = s_sb[:, o_bhn:o_bhn + 128]

            pos_half = r_sb[:, 0:SB]
            rmask = r_sb[:, SB:2 * SB]
            lin_b = r_sb[:, 2 * SB:2 * SB + 1]

            # ---- persistent state / stores ----
            # rz_store[:, s, :]: a1_r | xw2_r | a1_z | xw2_z
            rz_store = pp.tile([128, S, B4], f32, tag="rzs")
            # nx_store[:, s, :]: a1_n | xw2_n
            nx_store = pp.tile([128, S, B2], f32, tag="nxs")
            # hist[:, s, :]: [h1 | h2] state BEFORE slot s (fp32 carry);
            # hist16 is the fp16 shadow feeding the PE gate matmuls
            hist = pp.tile([128, S + 1, B2], f32, tag="hist")
            hist16 = pp.tile([128, S + 1, B2], f16, tag="hist16")
            nc.vector.memset(hist[:, 0:1, :], 0.0)
            nc.vector.memset(hist16[:, 0:1, :], 0.0)
            # layer-2 parts of store slots [0, DELTA): force z2 pad
            nc.vector.memset(rz_store[:, 0:DELTA, B:B2], 0.0)
            nc.vector.memset(rz_store[:, 0:DELTA, 3 * B:B4], -ZPAD)
            nc.vector.memset(nx_store[:, 0:DELTA, B:B2], 0.0)

            # ---- phase A: a1 = W_ih0 @ x (+ biases via selector MM) ----
            # emitted in 32-slot pieces: piece 0 up front, the rest spread
            # through the slot loop so they run in PE gaps
            APIECE = 32

            def phase_a_piece(j):
                t0 = APIECE * j
                t1 = min(t0 + APIECE, S)
                cols = (t1 - t0) * B
                c0 = t0 * B
                for g in range(3):
                    pa = psb.tile([128, 512], f32, tag="bulk")
                    nc.tensor.matmul(out=pa[:, 0:cols], lhsT=B1g(g),
                                     rhs=sel1[:, c0:c0 + cols],
                                     start=True, stop=False)
                    nc.tensor.matmul(out=pa[:, 0:cols], lhsT=Wih0_lo(g),
                                     rhs=x_sb[:, c0:c0 + cols],
                                     start=False, stop=False)
                    nc.tensor.matmul(out=pa[:, 0:cols], lhsT=Wih0_hi(g),
                                     rhs=x_sb[:, SB + c0:SB + c0 + cols],
                                     start=False, stop=True)
                    src = pa[:, 0:cols].rearrange("p (t b) -> p t b", b=B)
                    if g == 0:
                        nc.vector.tensor_copy(out=rz_store[:, t0:t1, 0:B],
                                              in_=src)
                    elif g == 1:
                        nc.vector.tensor_copy(out=rz_store[:, t0:t1,
                                                           2 * B:3 * B],
                                              in_=src)
                    else:
                        nc.vector.tensor_copy(out=nx_store[:, t0:t1, 0:B],
                                              in_=src)

            npieces = (S + APIECE - 1) // APIECE
            phase_a_piece(0)

            # ---- recurrence ----
            nchunks = (T + XCHUNK - 1) // XCHUNK
            next_chunk = 0

            for s in range(S):
                h_prev = hist[:, s, :]          # [128, 2B] = [h1 | h2]
                h1_prev = hist[:, s, 0:B]
                h2_prev = hist[:, s, B:B2]

                h1_16 = hist16[:, s, 0:B]
                h2_16 = hist16[:, s, B:B2]
                # bank R: [r1 r2]; bank Z (negated): [-z1 -z2]; bank N: [n1h n2h]
                pr = ps.tile([128, B2], f32, tag="gr")
                nc.tensor.matmul(out=pr[:, 0:B], lhsT=Whh0_16(0), rhs=h1_16,
                                 start=True, stop=False)
                nc.tensor.matmul(out=pr[:, B:B2], lhsT=Whh1_16(0), rhs=h2_16,
                                 start=False, stop=True)
                pz = ps.tile([128, B2], f32, tag="gz")
                nc.tensor.matmul(out=pz[:, 0:B], lhsT=Whh0_16(1), rhs=h1_16,
                                 start=True, stop=False)
                nc.tensor.matmul(out=pz[:, B:B2], lhsT=Whh1_16(1), rhs=h2_16,
                                 start=False, stop=True)
                pn = psn.tile([128, B2], f32, tag="nh")
                nc.tensor.matmul(out=pn[:], lhsT=bhnmat16, rhs=selnh16,
                                 start=True, stop=False)
                nc.tensor.matmul(out=pn[:, 0:B], lhsT=Whh0_16(2), rhs=h1_16,
                                 start=False, stop=False)
                nc.tensor.matmul(out=pn[:, B:B2], lhsT=Whh1_16(2), rhs=h2_16,
                                 start=False, stop=True)

                # pre-activations added in place in PSUM (sigmoid reads PSUM)
                nc.vector.tensor_tensor(out=pr[:], in0=pr[:],
                                        in1=rz_store[:, s, 0:B2], op=Alu.add)
                sig_r = wp.tile([128, B2], f32, tag="sigr")
                nc.scalar.activation(sig_r[:], pr[:], Act.Sigmoid)
                nc.vector.tensor_tensor(out=pz[:], in0=pz[:],
                                        in1=rz_store[:, s, B2:B4], op=Alu.add)
                # w = 1 - z = sigmoid(-pre_z); the z path is negated host-side
                w1z = wp.tile([128, B2], f32, tag="w1z")
                nc.scalar.activation(w1z[:], pz[:], Act.Sigmoid)
                tn = wp.tile([128, B2], f32, tag="tn")
                nc.vector.tensor_tensor(out=tn[:], in0=sig_r[:],
                                        in1=pn[:], op=Alu.mult)
                tn2 = psn.tile([128, B2], f32, tag="tn2")
                nc.vector.tensor_tensor(out=tn2[:], in0=tn[:],
                                        in1=nx_store[:, s, :], op=Alu.add)
                nt = wp.tile([128, B2], f32, tag="nt")
                nc.scalar.activation(nt[:], tn2[:], Act.Tanh)
                # v = z*h = h - w*h (off the critical chain; emitted after tn2)
                vwh = wp.tile([128, B2], f32, tag="vwh")
                nc.vector.tensor_tensor(out=vwh[:], in0=w1z[:],
                                        in1=h_prev, op=Alu.mult)
                vzh = wp.tile([128, B2], f32, tag="vzh")
                nc.vector.tensor_tensor(out=vzh[:], in0=h_prev,
                                        in1=vwh[:], op=Alu.subtract)
                uwn = wp.tile([128, B2], f32, tag="uwn")
                nc.vector.tensor_tensor(out=uwn[:], in0=w1z[:], in1=nt[:],
                                        op=Alu.mult)
                nc.vector.tensor_tensor(out=hist16[:, s + 1, :], in0=uwn[:],
                                        in1=vzh[:], op=Alu.add)
                nc.vector.tensor_tensor(out=hist[:, s + 1, :], in0=uwn[:],
                                        in1=vzh[:], op=Alu.add)

                # spread phase-A pieces into the loop (plenty of runway)
                if s >= 8 and (s - 8) % APIECE == 0:
                    j = (s - 8) // APIECE + 1
                    if j < npieces:
                        phase_a_piece(j)

                # bulk xw2 chunks, one gate per slot starting at 16c+15:
                # xw2_g = B2_16 @ sel2 + Wih1_hi @ h1 + 2^-10 * Wih1_lo_s @ h1
                for g in range(3):
                    cc = s - (XCHUNK - 1) - g
                    if cc < 0 or cc % XCHUNK != 0:
                        continue
                    c = cc // XCHUNK
                    if c >= nchunks:
                        continue
                    tp0 = XCHUNK * c
                    tp1 = min(tp0 + XCHUNK, T)
                    ccols = (tp1 - tp0) * B
                    px = psb.tile([128, 2, XCHUNK * B], f32, tag="bulk")
                    h1c16 = hist16[:, tp0 + 1:tp1 + 1, 0:B]
                    nc.tensor.matmul(
                        out=px[:, 0, 0:ccols], lhsT=B2g16(g),
                        rhs=sel2_16[:, tp0 * B:tp0 * B + ccols],
                        start=True, stop=False)
                    nc.tensor.matmul(out=px[:, 0, 0:ccols],
                                     lhsT=Wih1_hi16(g), rhs=h1c16,
                                     start=False, stop=True)
                    nc.tensor.matmul(out=px[:, 1, 0:ccols],
                                     lhsT=Wih1_lo16(g), rhs=h1c16,
                                     start=True, stop=True)
                    hicp = wp.tile([128, XCHUNK * B], f32, tag="hicp")
                    nc.vector.tensor_copy(out=hicp[:, 0:ccols],
                                          in_=px[:, 0, 0:ccols])
                    store, col = ((rz_store, B), (rz_store, 3 * B),
                                  (nx_store, B))[g]
                    nc.vector.scalar_tensor_tensor(
                        out=store[:, tp0 + DELTA:tp1 + DELTA, col:col + B],
                        in0=px[:, 1, 0:ccols].rearrange("p (t b) -> p t b",
                                                        b=B),
                        scalar=float(2.0 ** -10),
                        in1=hicp[:, 0:ccols].rearrange("p (t b) -> p t b",
                                                       b=B),
                        op0=Alu.mult, op1=Alu.add)

            # ---- scoring ----
            h2f = hist[:, S, B:B2]                     # final layer-2 states
            pbeta_t = psb.tile([1, 512], f32, tag="bulk")
            pbeta = pbeta_t[:, 0:B]
            nc.tensor.matmul(out=pbeta[:], lhsT=wh, rhs=h2f,
                             start=True, stop=True)
            beta = wp.tile([1, B], f32, tag="betasb")
            nc.vector.tensor_copy(out=beta[:], in_=pbeta[:])

            sc = pp.tile([1, S, B], f32, tag="sc")
            c0 = 0
            while c0 < SB:
                c1 = min(c0 + 512, SB)
                pscore = psb.tile([1, 512], f32, tag="bulk")
                nc.tensor.matmul(out=pscore[:, 0:c1 - c0], lhsT=ws_lo,
                                 rhs=x_sb[:, c0:c1], start=True, stop=False)
                nc.tensor.matmul(out=pscore[:, 0:c1 - c0], lhsT=ws_hi,
                                 rhs=x_sb[:, SB + c0:SB + c1],
                                 start=False, stop=True)
                nc.vector.tensor_copy(
                    out=sc[:].rearrange("p t b -> p (t b)")[:, c0:c1],
                    in_=pscore[:, 0:c1 - c0])
                c0 = c1

            # + beta[cluster], tanh(. + lin_b), mask, segment sums
            for b in range(B):
                nc.vector.tensor_scalar_add(out=sc[:, :, b:b + 1],
                                            in0=sc[:, :, b:b + 1],
                                            scalar1=beta[:, b:b + 1])
            th = pp.tile([1, S, B], f32, tag="th")
            nc.scalar.activation(th[:], sc[:], Act.Tanh, bias=lin_b)
            nc.vector.tensor_tensor(
                out=th[:], in0=th[:],
                in1=rmask.rearrange("p (t b) -> p t b", b=B), op=Alu.mult)
            sums = wp.tile([1, B], f32, tag="sums")
            for b in range(B):
                nc.vector.tensor_reduce(out=sums[:, b:b + 1],
                                        in_=th[:, :, b:b + 1],
                                        axis=mybir.AxisListType.XY, op=Alu.add)
            rsum = wp.tile([1, B], f32, tag="rsum")
            nc.vector.reciprocal(out=rsum[:], in_=sums[:])
            shalf = wp.tile([1, B], f32, tag="shalf")
            nc.vector.tensor_scalar_mul(out=shalf[:], in0=rsum[:],
                                        scalar1=P_SAL)
            fin = pp.tile([1, S, B], f32, tag="fin")
            for b in range(B):
                nc.vector.scalar_tensor_tensor(
                    out=fin[:, :, b:b + 1], in0=th[:, :, b:b + 1],
                    scalar=shalf[:, b:b + 1],
                    in1=pos_half.rearrange("p (t b) -> p t b", b=B)[:, :, b:b + 1],
                    op0=Alu.mult, op1=Alu.add)

            nc.sync.dma_start(out_dram.ap()[:],
                              fin[:].rearrange("p t b -> p (t b)"))

    nc.compile()
    return nc


def _prep_host(inputs):
    """Host-side sharding/packing.  Returns (T, S, in_maps, scatter)."""
    sent = np.ascontiguousarray(inputs["sent_gae_embeds"], dtype=np.float32)
    labels = np.asarray(inputs["labels"]).astype(np.int64)
    W_ih0 = np.asarray(inputs["W_ih0"], dtype=np.float32)
    W_hh0 = np.asarray(inputs["W_hh0"], dtype=np.float32)
    b_ih0 = np.asarray(inputs["b_ih0"], dtype=np.float32)
    b_hh0 = np.asarray(inputs["b_hh0"], dtype=np.float32)
    W_ih1 = np.asarray(inputs["W_ih1"], dtype=np.float32)
    W_hh1 = np.asarray(inputs["W_hh1"], dtype=np.float32)
    b_ih1 = np.asarray(inputs["b_ih1"], dtype=np.float32)
    b_hh1 = np.asarray(inputs["b_hh1"], dtype=np.float32)
    lin_v = np.asarray(inputs["lin_v"], dtype=np.float32)
    lin_g = np.asarray(inputs["lin_g"], dtype=np.float32)
    lin_b = np.asarray(inputs["lin_b"], dtype=np.float32)

    idx = [np.flatnonzero(labels == k) for k in range(K)]
    lens = np.array([max(len(i), 1) for i in idx])
    T = int(lens.max())
    S = T + DELTA
    SB = S * B

    # weight-normed linear
    Wn = (lin_g * lin_v / np.linalg.norm(lin_v, axis=1, keepdims=True))[0]
    w_s, w_h = Wn[:D], Wn[D:]

    # position scores (host constant table, permuted to cluster layout)
    inv = np.float32(1.0 / (N ** (1.0 / 3.0)))
    pos_full = np.maximum(np.float32(0.5),
                          np.exp(-(np.arange(N, dtype=np.float32) + 1.0) * inv))

    # shared weight pack
    WCOLS = 2 * 384 + 3 * 384 + 128 + 5
    gsign0 = np.ones(384, np.float32)
    gsign0[H:2 * H] = -1.0
    wpack = np.zeros((128, WCOLS), np.float32)
    wihT = W_ih0.T * gsign0[None, :]                 # [256, 384], z negated
    wpack[:, 0:384] = wihT[:128]
    wpack[:, 384:768] = wihT[128:]
    wpack[:, 768:1152] = W_hh0.T
    wpack[:, 1152:1536] = W_ih1.T * gsign0[None, :]
    wpack[:, 1536:1920] = W_hh1.T
    wpack[:, 1920:2048] = np.eye(128, dtype=np.float32)
    wpack[:, 2048] = w_s[:128]
    wpack[:, 2049] = w_s[128:]
    wpack[:, 2050] = w_h
    wpack[:, 2051] = b_hh0[2 * H:]
    wpack[:, 2052] = b_hh1[2 * H:]

    SCOLS = SB + T * B + 2 * B + 384 + 384 + 128
    o_sel2 = SB
    o_selnh = SB + T * B
    o_B1 = o_selnh + 2 * B
    o_B2 = o_B1 + 384
    o_bhn = o_B2 + 384

    spack_base = np.zeros((2, SCOLS), np.float32)
    # selnh: row0 -> first B cols (layer1), row1 -> second B cols.  (unused
    # now that bhn columns live in rz_store, but kept for layout stability)
    spack_base[0, o_selnh:o_selnh + B] = 1.0
    spack_base[1, o_selnh + B:o_selnh + 2 * B] = 1.0
    # B1: row0 = real bias (b_ih0 + b_hh0 for r,z; b_ih0 for n); row1 = pad
    breal = b_ih0.copy()
    breal[:2 * H] += b_hh0[:2 * H]
    breal[H:2 * H] *= -1.0
    spack_base[0, o_B1:o_B1 + 384] = breal
    spack_base[1, o_B1 + H:o_B1 + 2 * H] = -ZPAD
    breal2 = b_ih1.copy()
    breal2[:2 * H] += b_hh1[:2 * H]
    breal2[H:2 * H] *= -1.0
    spack_base[0, o_B2:o_B2 + 384] = breal2
    spack_base[1, o_B2 + H:o_B2 + 2 * H] = -ZPAD
    spack_base[0, o_bhn:o_bhn + 128] = b_hh0[2 * H:]
    spack_base[1, o_bhn:o_bhn + 128] = b_hh1[2 * H:]

    # The z-gate (gate 1) is negated everywhere so w = 1-z comes straight
    # out of sigmoid: sigma(-pre_z).
    gsign = np.ones(384, np.float32)
    gsign[H:2 * H] = -1.0
    w16pack = np.zeros((128, 1536), np.float16)
    w16pack[:, 0:384] = (W_hh0.T * gsign[None, :]).astype(np.float16)
    w16pack[:, 384:768] = (W_hh1.T * gsign[None, :]).astype(np.float16)
    wih1s = W_ih1.T * gsign[None, :]
    wih1_hi = wih1s.astype(np.float16)
    w16pack[:, 768:1152] = wih1_hi
    w16pack[:, 1152:1536] = ((wih1s - wih1_hi.astype(np.float32))
                             * 1024.0).astype(np.float16)
    s16base = np.zeros((2, 128 + 2 * B + 384 + T * B), np.float16)
    s16base[0, 0:128] = b_hh0[2 * H:].astype(np.float16)
    s16base[1, 0:128] = b_hh1[2 * H:].astype(np.float16)
    s16base[0, 128:128 + B] = 1.0
    s16base[1, 128 + B:128 + 2 * B] = 1.0
    s16base[:, 128 + 2 * B:128 + 2 * B + 384] = \
        spack_base[:, o_B2:o_B2 + 384].astype(np.float16)

    in_maps = []
    scatter = []  # per core: list of (orig_index, col) pairs
    for d in range(NCORES):
        xp = np.zeros((128, 2 * SB), np.float32)
        sp = spack_base.copy()
        rp = np.zeros((1, 2 * SB + 1), np.float32)
        sc_pairs = []
        for b in range(B):
            k = d * B + b
            ids = idx[k]
            L = lens[k]
            pad = T - L
            # real slots t in [pad, pad+L); sentence j = t - pad
            cols = (np.arange(pad, pad + L) * B + b)
            if len(ids):
                xp[:, cols] = sent[ids, :128].T
                xp[:, SB + cols] = sent[ids, 128:].T
                rp[0, cols] = (1.0 - P_SAL) * pos_full[ids]
                sc_pairs.append((ids, cols))
            rp[0, SB + cols] = 1.0                       # mask
            sp[0, cols] = 1.0                            # sel1 real
            sp[1, cols] = 0.0
            pads1 = np.concatenate([np.arange(0, pad), np.arange(pad + L, S)])
            sp[1, pads1 * B + b] = 1.0
            # sel2 indexed by t' in [0, T)
            c2 = o_sel2 + np.arange(T) * B + b
            sp[0, c2[pad:pad + L]] = 1.0
            sp[1, c2[:pad]] = 1.0
            sp[1, c2[pad + L:]] = 1.0
        rp[0, 2 * SB] = lin_b[0]
        s16 = s16base.copy()
        s16[:, 128 + 2 * B + 384:] = sp[:, o_sel2:o_sel2 + T * B].astype(
            np.float16)
        in_maps.append({"x_pack": xp, "w_pack": wpack, "s_pack": sp,
                        "r_pack": rp, "w16_pack": w16pack,
                        "s16_pack": s16})
        scatter.append(sc_pairs)

    return T, S, in_maps, scatter


_PROGRAM_CACHE = {}


def _install_ntff_hook_shim():
    """Provide antenv.axon_hooks (absent in this image) so that
    run_bass_kernel_spmd(trace=True) can capture NTFF profiles via the
    axon PJRT sidechannel.  Bench-only; never used by the grading path."""
    import contextlib
    import ctypes
    import types

    if "antenv.axon_hooks" in sys.modules:
        return
    so_path = "/opt/axon/libaxon_pjrt.so"
    hook = None
    if os.path.exists(so_path):
        lib = ctypes.CDLL(so_path)
        if hasattr(lib, "axon_start_nrt_profile"):
            lib.axon_start_nrt_profile.argtypes = [
                ctypes.POINTER(ctypes.c_int64), ctypes.c_size_t]
            lib.axon_start_nrt_profile.restype = ctypes.c_int64
            lib.axon_stop_nrt_profile.argtypes = [ctypes.c_char_p]
            lib.axon_stop_nrt_profile.restype = ctypes.c_int64

            @contextlib.contextmanager
            def _hook(output_dir, device_ids):
                import jax
                jax.devices()
                if device_ids:
                    ids = (ctypes.c_int64 * len(device_ids))(*device_ids)
                    rc = lib.axon_start_nrt_profile(ids, len(device_ids))
                else:
                    rc = lib.axon_start_nrt_profile(None, 0)
                if rc != 0:
                    raise RuntimeError(f"axon_start_nrt_profile rc={rc}")
                try:
                    yield
                finally:
                    n = lib.axon_stop_nrt_profile(str(output_dir).encode())
                    print(f"profile: {n} file(s) written to {output_dir}",
                          file=sys.stderr)

            hook = _hook

    mod = types.ModuleType("antenv.axon_hooks")
    mod.get_axon_ntff_profile_hook = lambda: hook
    mod.set_axon_ntff_profile_hook = lambda h: None
    sys.modules["antenv.axon_hooks"] = mod


def kernel(_bench=None, **inputs):
    from concourse import bass_utils

    if _bench is not None:
        _install_ntff_hook_shim()

    T, S, in_maps, scatter = _prep_host(inputs)

    key = (T, S)
    if key not in _PROGRAM_CACHE:
        _PROGRAM_CACHE[key] = _build_program(T, S)
    nc = _PROGRAM_CACHE[key]

    res = bass_utils.run_bass_kernel_spmd(
        nc, in_maps, core_ids=list(range(NCORES)),
        trace=_bench is not None, **(_bench or {}))

    out = np.zeros(N, np.float32)
    for d in range(NCORES):
        vals = res.results[d]["scores"][0]
        for ids, cols in scatter[d]:
            out[ids] = vals[cols]

    if _bench is not None:
        kernel._last_results = res
    return out
